# revision 26
# baseline (speedup 1.0000x reference)
"""Trainium2 Bass kernel for CAttention (contextual attention), v2.

Math (per batch element, derived from the reference):
    x:    (c=128, h=64, w=64), flat (128, 4096); m: (1, 4096)
    k    = normalize_rows(x.reshape(c, hw).T + eps)          # (4096, 128)
    y    = 3x3 zero-padded box filter of x                   # (128, 4096)
    S    = k @ y                                             # (4096 l, 4096 ij)
    att  = softmax over l (per column); u = exp(S - 20) (S bounded, col max
           >= ~11, so a constant shift suffices; att = u / colsum(u))
    rec  = k.T @ att                                         # (128, 4096)
    out  = rec * (1-m)/9 + x*m

Sharding: pure data parallel over batch (4) x output-column halves (2) = 8
cores, zero cross-core communication.

v2 structure (vs v1): per-core row-rotated x so the y-filter slab is always
columns [0:2176) of xb (no separate xyh input; wrapped pad row zeroed via a
tiny per-core mask input); xt shipped as fp8e4 (norms + kn only); ACT engine
is exp-pure (no Square-table thrash); column-sum tree is uniform groups of 4
(pair adds on DVE, second-level adds on GPSIMD which is kept OUT of the
u-tile recycling path, last group emitted directly); kn/xm/w_t on GPSIMD.
"""

import numpy as np
import ml_dtypes

NPBF16 = ml_dtypes.bfloat16
NPFP8 = ml_dtypes.float8_e4m3fn

SHIFT = 20.0
LN2_INV_128 = 128.0 / float(np.log(2.0))   # 184.6617
SCHR_C = 5.5
SCHR_OFF = 16256.0 - SHIFT * LN2_INV_128 - SCHR_C
C = 128          # channels
L = 4096         # spatial locations (l axis)
HALF = 2048      # output columns per core
BLK = 1024       # ij block (psum-bank sized: 2 banks)
NLT = 32         # l tiles of 128
YW = 2176        # y slab width: 34 rotated image rows x 64

# exp tiles handled by DVE (i16 Schraudolph); the rest go to ACT.
DVE_TILES = [{5, 13, 21},
             {3, 11, 19, 27}]

_CACHE = {}


def _build_program():
    import concourse.bass as bass
    import concourse.bacc as bacc
    import concourse.tile as tile
    import concourse.mybir as mybir

    F32 = mybir.dt.float32
    BF = mybir.dt.bfloat16
    FP8 = mybir.dt.float8e4
    I16 = mybir.dt.int16
    I32 = mybir.dt.int32
    AF = mybir.ActivationFunctionType
    ALU = mybir.AluOpType

    nc = bacc.Bacc("TRN2", target_bir_lowering=False, num_swdge_queues=4)

    xb_d = nc.dram_tensor("xb", [C, L], BF, kind="ExternalInput")
    # xt pre-tiled on host to SBUF layout: xt[p, t*128+c] = x[c, t*128+p]
    xt_d = nc.dram_tensor("xt", [C, L], FP8, kind="ExternalInput")
    mrep_d = nc.dram_tensor("mrep", [C, HALF], BF, kind="ExternalInput")
    ym_d = nc.dram_tensor("ym", [C, 2], F32, kind="ExternalInput")
    out_d = nc.dram_tensor("out", [C, HALF], F32, kind="ExternalOutput")

    with tile.TileContext(nc) as tc:
        with (
            tc.tile_pool(name="big", bufs=1) as big,
            tc.tile_pool(name="small", bufs=1) as small,
            tc.tile_pool(name="sqs", bufs=2) as sqs,
            tc.tile_pool(name="upool", bufs=10) as upool,
            tc.tile_pool(name="vpool", bufs=6) as vpool,
            tc.tile_pool(name="wpool", bufs=4) as wpool,
            tc.tile_pool(name="opool", bufs=4) as opool,
            tc.tile_pool(name="ps_sc", bufs=2, space=bass.MemorySpace.PSUM) as ps_sc,
            tc.tile_pool(name="ps_rec", bufs=1, space=bass.MemorySpace.PSUM) as ps_rec,
            tc.tile_pool(name="ps_sum", bufs=1, space=bass.MemorySpace.PSUM) as ps_sum,
        ):
            # ---- persistent SBUF tensors ----
            xb_sb = big.tile([C, L], BF, tag="xb_sb")      # mm1 stationary (c,l)
            xt_sb = big.tile([C, L], FP8, tag="xt_sb")     # l-major tiles (l,c)
            kn = big.tile([C, L], BF, tag="kn")            # normalized k, l-major
            y1 = big.tile([C, YW], BF, tag="y1")
            y_t = big.tile([C, HALF], BF, tag="y_t")
            mrep_sb = big.tile([C, HALF], BF, tag="mrep_sb")
            w_t = big.tile([C, HALF], F32, tag="w_t")      # (1-m)/9
            xm = big.tile([C, HALF], F32, tag="xm")        # x*m
            ones_t = small.tile([C, C], BF, tag="ones_t")
            ym_sb = small.tile([C, 2], F32, tag="ym_sb")
            norm2 = small.tile([C, NLT], F32, tag="norm2")
            rs_a = small.tile([C, NLT], F32, tag="rs_a")
            rs_b = small.tile([C, NLT], F32, tag="rs_b")
            nt_a = small.tile([C, NLT], F32, tag="nt_a")
            rs184 = small.tile([C, NLT], F32, tag="rs184")
            shift_c = small.tile([C, 1], F32, tag="shift_c")
            warm2 = small.tile([C, 1], F32, tag="warm2")

            # ---- input DMAs: only SP (sync) and ACT (scalar) have HW DMA
            # queues (~78GB/s each); order chunks by when they are needed.
            nc.sync.dma_start(xb_sb[:, 0:704], xb_d[:, 0:704])
            nc.scalar.dma_start(ym_sb[:], ym_d[:])
            nc.scalar.dma_start(xt_sb[:, 0:1024], xt_d[:, 0:1024])
            nc.sync.dma_start(xb_sb[:, 704:1280], xb_d[:, 704:1280])
            nc.scalar.dma_start(xt_sb[:, 1024:2048], xt_d[:, 1024:2048])
            nc.sync.dma_start(xb_sb[:, 1280:2176], xb_d[:, 1280:2176])
            nc.scalar.dma_start(xt_sb[:, 2048:L], xt_d[:, 2048:L])
            nc.sync.dma_start(xb_sb[:, 2176:3072], xb_d[:, 2176:3072])
            nc.sync.dma_start(xb_sb[:, 3072:L], xb_d[:, 3072:L])
            nc.scalar.dma_start(mrep_sb[:], mrep_d[:])

            # ---- tiny prologue constants ----
            nc.vector.memset(ones_t[:], 1.0)
            nc.vector.memset(shift_c[:], -SHIFT)

            # norm2[l] = sum_c xt[l, c]^2. ACT (Square+accum) for the first 8
            # tiles during the DMA head (frees the DVE head chain; costs one
            # extra Exp table reload paid before the exp stream starts); DVE
            # stt for the rest.
            def sq_chunk_act(l0, l1):
                for lt in range(l0, l1):
                    scr = sqs.tile([C, C], BF, tag="sq_scratch")
                    nc.scalar.activation(
                        scr[:], xt_sb[:, lt * C:(lt + 1) * C], AF.Square,
                        accum_out=norm2[:, lt:lt + 1])

            def sq_chunk(l0, l1):
                for lt in range(l0, l1):
                    scr = sqs.tile([C, C], BF, tag="sq_scratch")
                    nc.vector.scalar_tensor_tensor(
                        scr[:], xt_sb[:, lt * C:(lt + 1) * C], 1.0,
                        xt_sb[:, lt * C:(lt + 1) * C],
                        op0=ALU.mult, op1=ALU.mult,
                        accum_out=norm2[:, lt:lt + 1])

            # rsqrt via bit-trick seed + 2 Newton iterations (DVE, f32)
            rs_fin = rs_a

            def newton_chunk(l0, l1):
                cl = slice(l0, l1)
                nc.vector.tensor_scalar(nt_a[:, cl].bitcast(I32),
                                        norm2[:, cl].bitcast(I32), 1, None,
                                        op0=ALU.logical_shift_right)
                nc.vector.tensor_scalar(rs_a[:, cl].bitcast(I32),
                                        nt_a[:, cl].bitcast(I32),
                                        -1, 0x5f3759df,
                                        op0=ALU.mult, op1=ALU.add)
                src, dst = rs_a, rs_b
                for _ in range(2):
                    nc.vector.tensor_mul(nt_a[:, cl], src[:, cl], src[:, cl])
                    nc.vector.tensor_mul(nt_a[:, cl], nt_a[:, cl], norm2[:, cl])
                    nc.vector.tensor_scalar(nt_a[:, cl], nt_a[:, cl], -0.5, 1.5,
                                            op0=ALU.mult, op1=ALU.add)
                    nc.vector.tensor_mul(dst[:, cl], src[:, cl], nt_a[:, cl])
                    src, dst = dst, src
                nc.vector.tensor_scalar_mul(rs184[:, cl], rs_fin[:, cl],
                                            LN2_INV_128)

            def kn_chunk(l0, l1, eng):
                for lt in range(l0, l1):
                    eng.tensor_scalar_mul(
                        kn[:, lt * C:(lt + 1) * C], xt_sb[:, lt * C:(lt + 1) * C],
                        rs_fin[:, lt:lt + 1])

            # y = 3x3 box filter (row filter on xb cols [0:2176) -> y1, then
            # col filter over rotated-row positions; wrapped pad rows zeroed
            # by ym masks: pos 0 (maskA=0 iff h==0), pos 33 (maskB=0 iff h==1)
            xv = xb_sb[:, 0:YW].rearrange("p (r j) -> p r j", j=64)
            yv = y1[:].rearrange("p (r j) -> p r j", j=64)

            # --- critical-path-ordered prologue emission ---
            # ACT: squares for lt 0..8, Exp warm-up, then kn 0..8 via
            # Copy+scale (no activation table involved)
            sq_chunk_act(0, 8)
            nc.scalar.activation(warm2[:], shift_c[:], AF.Exp)
            # DVE: y part A: positions 0..9 -> y_t[0:512]
            nc.vector.tensor_add(y1[:, 1:639], xb_sb[:, 0:638],
                                 xb_sb[:, 1:639])
            nc.vector.tensor_add(y1[:, 1:639], y1[:, 1:639],
                                 xb_sb[:, 2:640])
            nc.vector.tensor_add(yv[:, 0:10, 0:1], xv[:, 0:10, 0:1],
                                 xv[:, 0:10, 1:2])
            nc.vector.tensor_add(yv[:, 0:10, 63:64], xv[:, 0:10, 62:63],
                                 xv[:, 0:10, 63:64])
            nc.vector.tensor_scalar_mul(y1[:, 0:64], y1[:, 0:64],
                                        ym_sb[:, 0:1])
            nc.vector.tensor_add(y_t[:, 0:512], y1[:, 0:512],
                                 y1[:, 64:64 + 512])
            nc.vector.tensor_add(y_t[:, 0:512], y_t[:, 0:512],
                                 y1[:, 128:128 + 512])
            # DVE: newton for lt 0..8 (norm2 from ACT squares) so the first
            # exps have their scale as soon as sc lands
            newton_chunk(0, 8)
            # DVE: y part B: positions 10..18 (interior flats [641:1216))
            nc.vector.tensor_add(y1[:, 641:1216], xb_sb[:, 640:1215],
                                 xb_sb[:, 641:1216])
            nc.vector.tensor_add(y1[:, 641:1216], y1[:, 641:1216],
                                 xb_sb[:, 642:1217])
            nc.vector.tensor_add(yv[:, 10:19, 0:1], xv[:, 10:19, 0:1],
                                 xv[:, 10:19, 1:2])
            nc.vector.tensor_add(yv[:, 10:19, 63:64], xv[:, 10:19, 62:63],
                                 xv[:, 10:19, 63:64])
            nc.vector.tensor_add(y_t[:, 512:BLK], y1[:, 512:BLK],
                                 y1[:, 512 + 64:64 + BLK])
            nc.vector.tensor_add(y_t[:, 512:BLK], y_t[:, 512:BLK],
                                 y1[:, 512 + 128:128 + BLK])
            kn_chunk(0, 8, nc.vector)

            def emit_exp(u, sc, lt, eng):
                if eng == "A":
                    nc.scalar.activation(u[:], sc[:], AF.Exp,
                                         bias=shift_c[:],
                                         scale=rs_fin[:, lt:lt + 1])
                else:
                    nc.vector.tensor_scalar(u[:].bitcast(I16), sc[:],
                                            rs184[:, lt:lt + 1], SCHR_OFF,
                                            op0=ALU.mult, op1=ALU.add)

            # ---- main loop: one global software pipeline over g = blk*32+lt
            # PE stream per slot: mm1_{g+1} then mm2_{g-1}; the exp for g runs
            # concurrently, so the ACT exp stream never waits on mm1 and the
            # DVE-assigned tiles overlap ACT tiles instead of stalling them.
            # sums tree per block: 8 groups of 4 lt tiles; groups 0..6 -> two
            # pair adds (DVE) + one L2 add (GPSIMD, off the u path) -> ones,
            # batched in pairs; group 7 -> pair sums emitted directly.
            N_ONES = 9
            NG = 2 * NLT
            st = {}   # per-block state

            def emit_mm1(g):
                blk, lt = divmod(g, NLT)
                sc = ps_sc.tile([C, BLK], F32, tag="sc", name="sc")
                for h2 in range(2):
                    cs = blk * BLK + h2 * 512
                    nc.tensor.matmul(
                        sc[:, h2 * 512:(h2 + 1) * 512],
                        xb_sb[:, lt * C:(lt + 1) * C],
                        y_t[:, cs:cs + 512],
                        start=True, stop=True,
                    )
                return sc

            def emit_mm2(g):
                blk, lt = divmod(g, NLT)
                s = st[blk]
                u = s["u"][lt]
                for h2 in range(2):
                    nc.tensor.matmul(
                        s["rec"][h2][:], kn[:, lt * C:(lt + 1) * C],
                        u[:, h2 * 512:(h2 + 1) * 512],
                        start=(lt == 0), stop=(lt == NLT - 1),
                    )

            def emit_ones(blk, w):
                s = st[blk]
                for h2 in range(2):
                    nc.tensor.matmul(
                        s["sums"][h2][:],
                        ones_t[:],
                        w[:, h2 * 512:(h2 + 1) * 512],
                        start=(s["oi"] == 0), stop=(s["oi"] == N_ONES - 1),
                    )
                s["oi"] += 1

            def emit_epilogue(blk, nchunk):
                s = st[blk]
                csz = BLK // nchunk
                for ch in range(nchunk):
                    cs = blk * BLK + ch * csz
                    h2 = (ch * csz) // 512
                    o2 = slice(ch * csz - h2 * 512, (ch + 1) * csz - h2 * 512)
                    R = opool.tile([C, csz], F32, tag=f"R{csz}", name="R")
                    nc.vector.reciprocal_approx_fast(R[:], s["sums"][h2][:, o2])
                    Rm = opool.tile([C, csz], F32, tag=f"Rm{csz}", name="Rm")
                    nc.vector.tensor_mul(Rm[:], R[:], w_t[:, cs:cs + csz])
                    ob = opool.tile([C, csz], F32, tag=f"ob{csz}", name="ob")
                    nc.vector.tensor_mul(ob[:], s["rec"][h2][:, o2], Rm[:])
                    nc.vector.tensor_add(ob[:], ob[:], xm[:, cs:cs + csz])
                    eng = nc.sync if ch % 2 == 0 else nc.scalar
                    eng.dma_start(out_d[:, cs:cs + csz], ob[:])

            sc_q = {0: emit_mm1(0)}
            for g in range(NG):
                blk, lt = divmod(g, NLT)
                if lt == 0:
                    st[blk] = {
                        "rec": [ps_rec.tile([C, 512], F32, tag=f"rec{h}",
                                            name=f"rec{h}") for h in range(2)],
                        "sums": [ps_sum.tile([C, 512], F32, tag=f"sums{h}",
                                             name=f"sums{h}") for h in range(2)],
                        "oi": 0, "wq": [], "pair": {}, "u": {},
                    }
                # block-0 prologue interleaves, paced by DMA-chunk landings;
                # wait guards keep the scheduler's model from slotting them
                # ahead of the critical first newton/kn chain
                if g == 2:
                    with tc.tile_wait_until(0.017):
                        sq_chunk(8, 16)
                elif g == 4:
                    with tc.tile_wait_until(0.018):
                        newton_chunk(8, 16)
                elif g == 5:
                    with tc.tile_wait_until(0.019):
                        kn_chunk(8, 16, nc.vector)
                elif g == 7:
                    # y part C: positions 19..33 -> y_t block 1. The wait
                    # guard keeps the scheduler from slotting these long ops
                    # (whose xb chunk lands late) ahead of the newton/kn
                    # chain the first exps depend on.
                    with tc.tile_wait_until(0.026):
                        nc.vector.tensor_add(y1[:, 1216:YW - 1],
                                             xb_sb[:, 1215:YW - 2],
                                             xb_sb[:, 1216:YW - 1])
                        nc.vector.tensor_add(y1[:, 1216:YW - 1],
                                             y1[:, 1216:YW - 1],
                                             xb_sb[:, 1217:YW])
                        nc.vector.tensor_add(yv[:, 19:34, 0:1],
                                             xv[:, 19:34, 0:1],
                                             xv[:, 19:34, 1:2])
                        nc.vector.tensor_add(yv[:, 19:34, 63:64],
                                             xv[:, 19:34, 62:63],
                                             xv[:, 19:34, 63:64])
                        nc.vector.tensor_scalar_mul(y1[:, 2112:YW],
                                                    y1[:, 2112:YW],
                                                    ym_sb[:, 1:2])
                        nc.vector.tensor_add(y_t[:, BLK:HALF],
                                             y1[:, BLK:BLK + BLK],
                                             y1[:, BLK + 64:BLK + 64 + BLK])
                        nc.vector.tensor_add(y_t[:, BLK:HALF],
                                             y_t[:, BLK:HALF],
                                             y1[:, BLK + 128:BLK + 128 + BLK])
                elif g == 8:
                    with tc.tile_wait_until(0.021):
                        sq_chunk(16, 24)
                elif g == 10:
                    with tc.tile_wait_until(0.022):
                        newton_chunk(16, 24)
                elif g == 11:
                    with tc.tile_wait_until(0.023):
                        kn_chunk(16, 24, nc.vector)
                elif g == 13:
                    with tc.tile_wait_until(0.029):
                        sq_chunk(24, 32)
                elif g == 15:
                    with tc.tile_wait_until(0.030):
                        newton_chunk(24, 32)
                elif g == 16:
                    with tc.tile_wait_until(0.031):
                        kn_chunk(24, 32, nc.vector)
                elif g == 27:
                    # x*m / (1-m)/9 in the gap between block-0's last L2 add
                    # and block-1's first (GPSIMD is in-order; anywhere else
                    # this delays the ones chain)
                    nc.gpsimd.tensor_mul(xm[:], xb_sb[:, 64:64 + HALF],
                                         mrep_sb[:])
                elif g == 29:
                    nc.gpsimd.tensor_scalar(w_t[:], mrep_sb[:],
                                            -1.0 / 9.0, 1.0 / 9.0,
                                            op0=ALU.mult, op1=ALU.add)
                # PE: prefetch mm1 two slots ahead of the exp stream
                if g + 1 < NG:
                    sc_q[g + 1] = emit_mm1(g + 1)
                # exp for g
                sc = sc_q.pop(g)
                u = upool.tile([C, BLK], BF, tag="u", name="u")
                emit_exp(u, sc, lt, "D" if lt in DVE_TILES[blk] else "A")
                s = st[blk]
                s["u"][lt] = u
                # PE: mm2 for the previous slot
                if g >= 1:
                    emit_mm2(g - 1)
                # column-sum tree for g
                if lt % 2 == 0:
                    s["pair"]["u"] = u
                else:
                    v = vpool.tile([C, BLK], BF, tag="v", name="v")
                    nc.vector.tensor_add(v[:], s["pair"].pop("u")[:], u[:])
                    if lt >= NLT - 4:
                        s["wq"].append(v)     # last group: pair sums direct
                    elif lt % 4 == 1:
                        s["pair"]["v1"] = v
                    else:
                        w = wpool.tile([C, BLK], BF, tag="w", name="w")
                        nc.gpsimd.tensor_add(w[:], s["pair"].pop("v1")[:], v[:])
                        s["wq"].append(w)
                        # batch ones emissions in pairs, lagged ~2 groups
                        if len(s["wq"]) > 2:
                            emit_ones(blk, s["wq"].pop(0))
                            emit_ones(blk, s["wq"].pop(0))
                # end-of-block drains ride the next block's pipeline slots
                if lt == NLT - 1:
                    if g + 1 >= NG:      # final block: drain immediately
                        emit_mm2(g)
                        for w in s["wq"]:
                            emit_ones(blk, w)
                        s["wq"] = []
                        emit_epilogue(blk, 4)
                elif lt == 0 and blk > 0:
                    pb = st[blk - 1]
                    for w in pb["wq"]:
                        emit_ones(blk - 1, w)
                    pb["wq"] = []
                elif lt == 1 and blk > 0:
                    emit_epilogue(blk - 1, 2)

    nc.finalize()
    return nc


def _get_program():
    if "nc" not in _CACHE:
        _CACHE["nc"] = _build_program()
    return _CACHE["nc"]


def _make_in_maps(fg, mk):
    in_maps = []
    for core in range(8):
        b, h = core // 2, core % 2
        start = 63 if h == 0 else 31   # rotated row order R[p] = (start+p)%64
        xi = np.roll(fg[b].reshape(C, 64, 64), -start, axis=1)
        x = np.ascontiguousarray(xi.reshape(C, L))
        xb = x.astype(NPBF16)
        # pre-tiled transpose: xt[p, t*128+c] = x[c, t*128+p]
        xt = np.ascontiguousarray(
            x.reshape(C, L // C, C).transpose(2, 1, 0).reshape(C, L)).astype(NPFP8)
        mi = np.roll(mk[b].reshape(1, 64, 64), -start, axis=1)
        mrow = mi.reshape(1, L)[:, 64:64 + HALF]
        mrep = np.ascontiguousarray(
            np.broadcast_to(mrow, (C, HALF))).astype(NPBF16)
        ym = np.empty((C, 2), np.float32)
        ym[:, 0] = 0.0 if h == 0 else 1.0
        ym[:, 1] = 0.0 if h == 1 else 1.0
        in_maps.append({"xb": xb, "xt": xt, "mrep": mrep, "ym": ym})
    return in_maps


def kernel(foreground, mask):
    fg = np.ascontiguousarray(np.asarray(foreground, dtype=np.float32))
    mk = np.ascontiguousarray(np.asarray(mask, dtype=np.float32))
    nc = _get_program()
    in_maps = _make_in_maps(fg, mk)

    from concourse.bass_utils import run_bass_kernel_spmd
    res = run_bass_kernel_spmd(nc, in_maps, core_ids=list(range(8)))

    out = np.empty((4, C, L), np.float32)
    for core in range(8):
        b, h = core // 2, core % 2
        # kernel columns = rotated positions 1..32 = image rows h*32..h*32+31
        out[b][:, h * HALF:(h + 1) * HALF] = res.results[core]["out"]
    return out.reshape(4, C, 64, 64)


# revision 34
# speedup vs baseline: 1.0291x; 1.0291x over previous
"""Trainium2 Bass kernel for CAttention (contextual attention), v2.

Math (per batch element, derived from the reference):
    x:    (c=128, h=64, w=64), flat (128, 4096); m: (1, 4096)
    k    = normalize_rows(x.reshape(c, hw).T + eps)          # (4096, 128)
    y    = 3x3 zero-padded box filter of x                   # (128, 4096)
    S    = k @ y                                             # (4096 l, 4096 ij)
    att  = softmax over l (per column); u = exp(S - 20) (S bounded, col max
           >= ~11, so a constant shift suffices; att = u / colsum(u))
    rec  = k.T @ att                                         # (128, 4096)
    out  = rec * (1-m)/9 + x*m

Sharding: pure data parallel over batch (4) x output-column halves (2) = 8
cores, zero cross-core communication.

v2 structure (vs v1): per-core row-rotated x so the y-filter slab is always
columns [0:2176) of xb (no separate xyh input; wrapped pad row zeroed via a
tiny per-core mask input); xt shipped as fp8e4 (norms + kn only); ACT engine
is exp-pure (no Square-table thrash); column-sum tree is uniform groups of 4
(pair adds on DVE, second-level adds on GPSIMD which is kept OUT of the
u-tile recycling path, last group emitted directly); kn/xm/w_t on GPSIMD.
"""

import numpy as np
import ml_dtypes

NPBF16 = ml_dtypes.bfloat16
NPFP8 = ml_dtypes.float8_e4m3fn

SHIFT = 20.0
LN2_INV_128 = 128.0 / float(np.log(2.0))   # 184.6617
SCHR_C = 5.5
SCHR_OFF = 16256.0 - SHIFT * LN2_INV_128 - SCHR_C
C = 128          # channels
L = 4096         # spatial locations (l axis)
HALF = 2048      # output columns per core
BLK = 1024       # ij block (psum-bank sized: 2 banks)
NLT = 32         # l tiles of 128
YW = 2176        # y slab width: 34 rotated image rows x 64

# exp tiles handled by DVE (i16 Schraudolph); the rest go to ACT.
DVE_TILES = [{5, 13, 21},
             {3, 11, 19, 27}]

_CACHE = {}


def _build_program():
    import concourse.bass as bass
    import concourse.bacc as bacc
    import concourse.tile as tile
    import concourse.mybir as mybir

    F32 = mybir.dt.float32
    BF = mybir.dt.bfloat16
    FP8 = mybir.dt.float8e4
    I16 = mybir.dt.int16
    I32 = mybir.dt.int32
    AF = mybir.ActivationFunctionType
    ALU = mybir.AluOpType

    nc = bacc.Bacc("TRN2", target_bir_lowering=False, num_swdge_queues=2)

    xb_d = nc.dram_tensor("xb", [C, L], BF, kind="ExternalInput")
    # xt pre-tiled on host to SBUF layout: xt[p, t*128+c] = x[c, t*128+p]
    xt_d = nc.dram_tensor("xt", [C, L], FP8, kind="ExternalInput")
    mrep_d = nc.dram_tensor("mrep", [C, HALF], BF, kind="ExternalInput")
    ym_d = nc.dram_tensor("ym", [C, 2], F32, kind="ExternalInput")
    out_d = nc.dram_tensor("out", [C, HALF], F32, kind="ExternalOutput")

    with tile.TileContext(nc) as tc:
        with (
            tc.tile_pool(name="big", bufs=1) as big,
            tc.tile_pool(name="small", bufs=1) as small,
            tc.tile_pool(name="sqs", bufs=2) as sqs,
            tc.tile_pool(name="work", bufs=10) as upool,
            tc.tile_pool(name="opool", bufs=4) as opool,
            tc.tile_pool(name="ps_sc", bufs=2, space=bass.MemorySpace.PSUM) as ps_sc,
            tc.tile_pool(name="ps_rec", bufs=1, space=bass.MemorySpace.PSUM) as ps_rec,
            tc.tile_pool(name="ps_sum", bufs=1, space=bass.MemorySpace.PSUM) as ps_sum,
        ):
            # ---- persistent SBUF tensors ----
            xb_sb = big.tile([C, L], BF, tag="xb_sb")      # mm1 stationary (c,l)
            xt_sb = big.tile([C, L], FP8, tag="xt_sb")     # l-major tiles (l,c)
            kn = big.tile([C, L], BF, tag="kn")            # normalized k, l-major
            y1 = big.tile([C, YW], BF, tag="y1")
            y_t = big.tile([C, HALF], BF, tag="y_t")
            mrep_sb = big.tile([C, HALF], BF, tag="mrep_sb")
            w_t = big.tile([C, HALF], F32, tag="w_t")      # (1-m)/9
            xm = big.tile([C, HALF], F32, tag="xm")        # x*m
            ones_t = small.tile([C, C], BF, tag="ones_t")
            ym_sb = small.tile([C, 2], F32, tag="ym_sb")
            norm2 = small.tile([C, NLT], F32, tag="norm2")
            rs_a = small.tile([C, NLT], F32, tag="rs_a")
            rs_b = small.tile([C, NLT], F32, tag="rs_b")
            nt_a = small.tile([C, NLT], F32, tag="nt_a")
            rs184 = small.tile([C, NLT], F32, tag="rs184")
            shift_c = small.tile([C, 1], F32, tag="shift_c")
            warm2 = small.tile([C, 1], F32, tag="warm2")

            # ---- input DMAs: only SP (sync) and ACT (scalar) have HW DMA
            # queues (~78GB/s each); order chunks by when they are needed.
            nc.sync.dma_start(xb_sb[:, 0:704], xb_d[:, 0:704])
            nc.scalar.dma_start(ym_sb[:], ym_d[:])
            nc.scalar.dma_start(xt_sb[:, 0:1024], xt_d[:, 0:1024])
            nc.sync.dma_start(xb_sb[:, 704:1280], xb_d[:, 704:1280])
            nc.scalar.dma_start(xt_sb[:, 1024:2048], xt_d[:, 1024:2048])
            nc.sync.dma_start(xb_sb[:, 1280:2176], xb_d[:, 1280:2176])
            nc.scalar.dma_start(xt_sb[:, 2048:L], xt_d[:, 2048:L])
            nc.sync.dma_start(xb_sb[:, 2176:3072], xb_d[:, 2176:3072])
            nc.sync.dma_start(xb_sb[:, 3072:L], xb_d[:, 3072:L])
            nc.scalar.dma_start(mrep_sb[:], mrep_d[:])

            # ---- tiny prologue constants ----
            nc.vector.memset(ones_t[:], 1.0)
            nc.vector.memset(shift_c[:], -SHIFT)

            # norm2[l] = sum_c xt[l, c]^2. ACT (Square+accum) for the first 8
            # tiles during the DMA head (frees the DVE head chain; costs one
            # extra Exp table reload paid before the exp stream starts); DVE
            # stt for the rest.
            def sq_chunk_act(l0, l1):
                for lt in range(l0, l1):
                    scr = sqs.tile([C, C], BF, tag="sq_scratch")
                    nc.scalar.activation(
                        scr[:], xt_sb[:, lt * C:(lt + 1) * C], AF.Square,
                        accum_out=norm2[:, lt:lt + 1])

            def sq_chunk(l0, l1):
                for lt in range(l0, l1):
                    scr = sqs.tile([C, C], BF, tag="sq_scratch")
                    nc.vector.scalar_tensor_tensor(
                        scr[:], xt_sb[:, lt * C:(lt + 1) * C], 1.0,
                        xt_sb[:, lt * C:(lt + 1) * C],
                        op0=ALU.mult, op1=ALU.mult,
                        accum_out=norm2[:, lt:lt + 1])

            # rsqrt via bit-trick seed + 2 Newton iterations (DVE, f32)
            rs_fin = rs_a

            def newton_chunk(l0, l1):
                cl = slice(l0, l1)
                nc.vector.tensor_scalar(nt_a[:, cl].bitcast(I32),
                                        norm2[:, cl].bitcast(I32), 1, None,
                                        op0=ALU.logical_shift_right)
                nc.vector.tensor_scalar(rs_a[:, cl].bitcast(I32),
                                        nt_a[:, cl].bitcast(I32),
                                        -1, 0x5f3759df,
                                        op0=ALU.mult, op1=ALU.add)
                src, dst = rs_a, rs_b
                for _ in range(2):
                    nc.vector.tensor_mul(nt_a[:, cl], src[:, cl], src[:, cl])
                    nc.vector.tensor_mul(nt_a[:, cl], nt_a[:, cl], norm2[:, cl])
                    nc.vector.tensor_scalar(nt_a[:, cl], nt_a[:, cl], -0.5, 1.5,
                                            op0=ALU.mult, op1=ALU.add)
                    nc.vector.tensor_mul(dst[:, cl], src[:, cl], nt_a[:, cl])
                    src, dst = dst, src
                nc.vector.tensor_scalar_mul(rs184[:, cl], rs_fin[:, cl],
                                            LN2_INV_128)

            def kn_chunk(l0, l1, eng):
                for lt in range(l0, l1):
                    eng.tensor_scalar_mul(
                        kn[:, lt * C:(lt + 1) * C], xt_sb[:, lt * C:(lt + 1) * C],
                        rs_fin[:, lt:lt + 1])

            # y = 3x3 box filter (row filter on xb cols [0:2176) -> y1, then
            # col filter over rotated-row positions; wrapped pad rows zeroed
            # by ym masks: pos 0 (maskA=0 iff h==0), pos 33 (maskB=0 iff h==1)
            xv = xb_sb[:, 0:YW].rearrange("p (r j) -> p r j", j=64)
            yv = y1[:].rearrange("p (r j) -> p r j", j=64)

            # --- critical-path-ordered prologue emission ---
            # ACT: squares for lt 0..8, Exp warm-up, then kn 0..8 via
            # Copy+scale (no activation table involved)
            sq_chunk_act(0, 8)
            nc.scalar.activation(warm2[:], shift_c[:], AF.Exp)
            # DVE: y part A: positions 0..9 -> y_t[0:512]
            nc.vector.tensor_add(y1[:, 1:639], xb_sb[:, 0:638],
                                 xb_sb[:, 1:639])
            nc.vector.tensor_add(y1[:, 1:639], y1[:, 1:639],
                                 xb_sb[:, 2:640])
            nc.vector.tensor_add(yv[:, 0:10, 0:1], xv[:, 0:10, 0:1],
                                 xv[:, 0:10, 1:2])
            nc.vector.tensor_add(yv[:, 0:10, 63:64], xv[:, 0:10, 62:63],
                                 xv[:, 0:10, 63:64])
            nc.vector.tensor_scalar_mul(y1[:, 0:64], y1[:, 0:64],
                                        ym_sb[:, 0:1])
            nc.vector.tensor_add(y_t[:, 0:512], y1[:, 0:512],
                                 y1[:, 64:64 + 512])
            nc.vector.tensor_add(y_t[:, 0:512], y_t[:, 0:512],
                                 y1[:, 128:128 + 512])
            # DVE: newton for lt 0..8 (norm2 from ACT squares) so the first
            # exps have their scale as soon as sc lands
            newton_chunk(0, 8)
            # DVE: y part B: positions 10..18 (interior flats [641:1216))
            nc.vector.tensor_add(y1[:, 641:1216], xb_sb[:, 640:1215],
                                 xb_sb[:, 641:1216])
            nc.vector.tensor_add(y1[:, 641:1216], y1[:, 641:1216],
                                 xb_sb[:, 642:1217])
            nc.vector.tensor_add(yv[:, 10:19, 0:1], xv[:, 10:19, 0:1],
                                 xv[:, 10:19, 1:2])
            nc.vector.tensor_add(yv[:, 10:19, 63:64], xv[:, 10:19, 62:63],
                                 xv[:, 10:19, 63:64])
            nc.vector.tensor_add(y_t[:, 512:BLK], y1[:, 512:BLK],
                                 y1[:, 512 + 64:64 + BLK])
            nc.vector.tensor_add(y_t[:, 512:BLK], y_t[:, 512:BLK],
                                 y1[:, 512 + 128:128 + BLK])
            kn_chunk(0, 8, nc.vector)

            def emit_exp(u, sc, lt, eng):
                if eng == "A":
                    nc.scalar.activation(u[:], sc[:], AF.Exp,
                                         bias=shift_c[:],
                                         scale=rs_fin[:, lt:lt + 1])
                else:
                    nc.vector.tensor_scalar(u[:].bitcast(I16), sc[:],
                                            rs184[:, lt:lt + 1], SCHR_OFF,
                                            op0=ALU.mult, op1=ALU.add)

            # ---- main loop: one global software pipeline over g = blk*32+lt
            # PE stream per slot: mm1_{g+1} then mm2_{g-1}; the exp for g runs
            # concurrently, so the ACT exp stream never waits on mm1 and the
            # DVE-assigned tiles overlap ACT tiles instead of stalling them.
            # sums tree per block: 8 groups of 4 lt tiles; groups 0..6 -> two
            # pair adds (DVE) + one L2 add (GPSIMD, off the u path) -> ones,
            # batched in pairs; group 7 -> pair sums emitted directly.
            N_ONES = 9
            NG = 2 * NLT
            st = {}   # per-block state

            def emit_mm1(g):
                blk, lt = divmod(g, NLT)
                sc = ps_sc.tile([C, BLK], F32, tag="sc", name="sc")
                for h2 in range(2):
                    cs = blk * BLK + h2 * 512
                    nc.tensor.matmul(
                        sc[:, h2 * 512:(h2 + 1) * 512],
                        xb_sb[:, lt * C:(lt + 1) * C],
                        y_t[:, cs:cs + 512],
                        start=True, stop=True,
                    )
                return sc

            def emit_mm2(g):
                blk, lt = divmod(g, NLT)
                s = st[blk]
                u = s["u"][lt]
                for h2 in range(2):
                    nc.tensor.matmul(
                        s["rec"][h2][:], kn[:, lt * C:(lt + 1) * C],
                        u[:, h2 * 512:(h2 + 1) * 512],
                        start=(lt == 0), stop=(lt == NLT - 1),
                    )

            def emit_ones(blk, w):
                s = st[blk]
                for h2 in range(2):
                    nc.tensor.matmul(
                        s["sums"][h2][:],
                        ones_t[:],
                        w[:, h2 * 512:(h2 + 1) * 512],
                        start=(s["oi"] == 0), stop=(s["oi"] == N_ONES - 1),
                    )
                s["oi"] += 1

            def emit_epilogue(blk, nchunk):
                s = st[blk]
                csz = BLK // nchunk
                for ch in range(nchunk):
                    cs = blk * BLK + ch * csz
                    h2 = (ch * csz) // 512
                    o2 = slice(ch * csz - h2 * 512, (ch + 1) * csz - h2 * 512)
                    R = opool.tile([C, csz], F32, tag=f"R{csz}", name="R")
                    nc.vector.reciprocal_approx_fast(R[:], s["sums"][h2][:, o2])
                    Rm = opool.tile([C, csz], F32, tag=f"Rm{csz}", name="Rm")
                    nc.vector.tensor_mul(Rm[:], R[:], w_t[:, cs:cs + csz])
                    ob = opool.tile([C, csz], F32, tag=f"ob{csz}", name="ob")
                    nc.vector.tensor_mul(ob[:], s["rec"][h2][:, o2], Rm[:])
                    nc.vector.tensor_add(ob[:], ob[:], xm[:, cs:cs + csz])
                    eng = nc.sync if ch % 2 == 0 else nc.scalar
                    eng.dma_start(out_d[:, cs:cs + csz], ob[:])

            sc_q = {0: emit_mm1(0)}
            for g in range(NG):
                blk, lt = divmod(g, NLT)
                if lt == 0:
                    st[blk] = {
                        "rec": [ps_rec.tile([C, 512], F32, tag=f"rec{h}",
                                            name=f"rec{h}") for h in range(2)],
                        "sums": [ps_sum.tile([C, 512], F32, tag=f"sums{h}",
                                             name=f"sums{h}") for h in range(2)],
                        "oi": 0, "wq": [], "pair": {}, "u": {},
                    }
                # block-0 prologue interleaves, paced by DMA-chunk landings;
                # wait guards keep the scheduler's model from slotting them
                # ahead of the critical first newton/kn chain
                if g == 2:
                    sq_chunk(8, 16)
                elif g == 4:
                    newton_chunk(8, 16)
                elif g == 5:
                    kn_chunk(8, 16, nc.vector)
                elif g == 7:
                    # y part C: positions 19..33 -> y_t block 1. The wait
                    # guard keeps the scheduler from slotting these long ops
                    # (whose xb chunk lands late) ahead of the newton/kn
                    # chain the first exps depend on.
                    with tc.tile_wait_until(0.024):
                        nc.vector.tensor_add(y1[:, 1216:YW - 1],
                                             xb_sb[:, 1215:YW - 2],
                                             xb_sb[:, 1216:YW - 1])
                        nc.vector.tensor_add(y1[:, 1216:YW - 1],
                                             y1[:, 1216:YW - 1],
                                             xb_sb[:, 1217:YW])
                        nc.vector.tensor_add(yv[:, 19:34, 0:1],
                                             xv[:, 19:34, 0:1],
                                             xv[:, 19:34, 1:2])
                        nc.vector.tensor_add(yv[:, 19:34, 63:64],
                                             xv[:, 19:34, 62:63],
                                             xv[:, 19:34, 63:64])
                        nc.vector.tensor_scalar_mul(y1[:, 2112:YW],
                                                    y1[:, 2112:YW],
                                                    ym_sb[:, 1:2])
                        nc.vector.tensor_add(y_t[:, BLK:HALF],
                                             y1[:, BLK:BLK + BLK],
                                             y1[:, BLK + 64:BLK + 64 + BLK])
                        nc.vector.tensor_add(y_t[:, BLK:HALF],
                                             y_t[:, BLK:HALF],
                                             y1[:, BLK + 128:BLK + 128 + BLK])
                elif g == 8:
                    sq_chunk(16, 24)
                elif g == 10:
                    newton_chunk(16, 24)
                elif g == 11:
                    kn_chunk(16, 24, nc.vector)
                elif g == 13:
                    sq_chunk(24, 32)
                elif g == 15:
                    newton_chunk(24, 32)
                elif g == 16:
                    kn_chunk(24, 32, nc.vector)
                elif g == 27:
                    # x*m / (1-m)/9 in the gap between block-0's last L2 add
                    # and block-1's first (GPSIMD is in-order; anywhere else
                    # this delays the ones chain)
                    nc.gpsimd.tensor_mul(xm[:], xb_sb[:, 64:64 + HALF],
                                         mrep_sb[:])
                elif g == 29:
                    nc.gpsimd.tensor_scalar(w_t[:], mrep_sb[:],
                                            -1.0 / 9.0, 1.0 / 9.0,
                                            op0=ALU.mult, op1=ALU.add)
                # PE: prefetch mm1 two slots ahead of the exp stream
                if g + 1 < NG:
                    sc_q[g + 1] = emit_mm1(g + 1)
                # exp for g
                sc = sc_q.pop(g)
                u = upool.tile([C, BLK], BF, tag="u", name="u")
                emit_exp(u, sc, lt, "D" if lt in DVE_TILES[blk] else "A")
                s = st[blk]
                s["u"][lt] = u
                # PE: mm2 for the previous slot
                if g >= 1:
                    emit_mm2(g - 1)
                # column-sum tree for g
                if lt % 2 == 0:
                    s["pair"]["u"] = u
                else:
                    v = upool.tile([C, BLK], BF, tag="v", name="v")
                    peng = nc.gpsimd if lt in (13, 21) else nc.vector
                    peng.tensor_add(v[:], s["pair"].pop("u")[:], u[:])
                    if lt >= NLT - 4:
                        s["wq"].append(v)     # last group: pair sums direct
                    elif lt % 4 == 1:
                        s["pair"]["v1"] = v
                    else:
                        w = upool.tile([C, BLK], BF, tag="w", name="w")
                        nc.gpsimd.tensor_add(w[:], s["pair"].pop("v1")[:], v[:])
                        s["wq"].append(w)
                        # batch ones emissions in pairs, lagged ~2 groups
                        if len(s["wq"]) > 2:
                            emit_ones(blk, s["wq"].pop(0))
                            emit_ones(blk, s["wq"].pop(0))
                # end-of-block drains ride the next block's pipeline slots
                if lt == NLT - 1:
                    if g + 1 >= NG:      # final block: drain immediately
                        emit_mm2(g)
                        for w in s["wq"]:
                            emit_ones(blk, w)
                        s["wq"] = []
                        emit_epilogue(blk, 4)
                elif lt == 0 and blk > 0:
                    pb = st[blk - 1]
                    for w in pb["wq"]:
                        emit_ones(blk - 1, w)
                    pb["wq"] = []
                elif lt == 1 and blk > 0:
                    emit_epilogue(blk - 1, 2)

    nc.finalize()
    return nc


def _get_program():
    if "nc" not in _CACHE:
        _CACHE["nc"] = _build_program()
    return _CACHE["nc"]


def _make_in_maps(fg, mk):
    in_maps = []
    for core in range(8):
        b, h = core // 2, core % 2
        start = 63 if h == 0 else 31   # rotated row order R[p] = (start+p)%64
        xi = np.roll(fg[b].reshape(C, 64, 64), -start, axis=1)
        x = np.ascontiguousarray(xi.reshape(C, L))
        xb = x.astype(NPBF16)
        # pre-tiled transpose: xt[p, t*128+c] = x[c, t*128+p]
        xt = np.ascontiguousarray(
            x.reshape(C, L // C, C).transpose(2, 1, 0).reshape(C, L)).astype(NPFP8)
        mi = np.roll(mk[b].reshape(1, 64, 64), -start, axis=1)
        mrow = mi.reshape(1, L)[:, 64:64 + HALF]
        mrep = np.ascontiguousarray(
            np.broadcast_to(mrow, (C, HALF))).astype(NPBF16)
        ym = np.empty((C, 2), np.float32)
        ym[:, 0] = 0.0 if h == 0 else 1.0
        ym[:, 1] = 0.0 if h == 1 else 1.0
        in_maps.append({"xb": xb, "xt": xt, "mrep": mrep, "ym": ym})
    return in_maps


def kernel(foreground, mask):
    fg = np.ascontiguousarray(np.asarray(foreground, dtype=np.float32))
    mk = np.ascontiguousarray(np.asarray(mask, dtype=np.float32))
    nc = _get_program()
    in_maps = _make_in_maps(fg, mk)

    from concourse.bass_utils import run_bass_kernel_spmd
    res = run_bass_kernel_spmd(nc, in_maps, core_ids=list(range(8)))

    out = np.empty((4, C, L), np.float32)
    for core in range(8):
        b, h = core // 2, core % 2
        # kernel columns = rotated positions 1..32 = image rows h*32..h*32+31
        out[b][:, h * HALF:(h + 1) * HALF] = res.results[core]["out"]
    return out.reshape(4, C, 64, 64)


# revision 35
# speedup vs baseline: 1.0358x; 1.0066x over previous
"""Trainium2 Bass kernel for CAttention (contextual attention), v2.

Math (per batch element, derived from the reference):
    x:    (c=128, h=64, w=64), flat (128, 4096); m: (1, 4096)
    k    = normalize_rows(x.reshape(c, hw).T + eps)          # (4096, 128)
    y    = 3x3 zero-padded box filter of x                   # (128, 4096)
    S    = k @ y                                             # (4096 l, 4096 ij)
    att  = softmax over l (per column); u = exp(S - 20) (S bounded, col max
           >= ~11, so a constant shift suffices; att = u / colsum(u))
    rec  = k.T @ att                                         # (128, 4096)
    out  = rec * (1-m)/9 + x*m

Sharding: pure data parallel over batch (4) x output-column halves (2) = 8
cores, zero cross-core communication.

v2 structure (vs v1): per-core row-rotated x so the y-filter slab is always
columns [0:2176) of xb (no separate xyh input; wrapped pad row zeroed via a
tiny per-core mask input); xt shipped as fp8e4 (norms + kn only); ACT engine
is exp-pure (no Square-table thrash); column-sum tree is uniform groups of 4
(pair adds on DVE, second-level adds on GPSIMD which is kept OUT of the
u-tile recycling path, last group emitted directly); kn/xm/w_t on GPSIMD.
"""

import numpy as np
import ml_dtypes

NPBF16 = ml_dtypes.bfloat16
NPFP8 = ml_dtypes.float8_e4m3fn

SHIFT = 20.0
LN2_INV_128 = 128.0 / float(np.log(2.0))   # 184.6617
SCHR_C = 5.5
SCHR_OFF = 16256.0 - SHIFT * LN2_INV_128 - SCHR_C
C = 128          # channels
L = 4096         # spatial locations (l axis)
HALF = 2048      # output columns per core
BLK = 1024       # ij block (psum-bank sized: 2 banks)
NLT = 32         # l tiles of 128
YW = 2176        # y slab width: 34 rotated image rows x 64

# exp tiles handled by DVE (i16 Schraudolph); the rest go to ACT.
DVE_TILES = [{5, 13, 21},
             {3, 11, 19, 27}]

_CACHE = {}


def _build_program():
    import concourse.bass as bass
    import concourse.bacc as bacc
    import concourse.tile as tile
    import concourse.mybir as mybir

    F32 = mybir.dt.float32
    BF = mybir.dt.bfloat16
    FP8 = mybir.dt.float8e4
    I16 = mybir.dt.int16
    I32 = mybir.dt.int32
    AF = mybir.ActivationFunctionType
    ALU = mybir.AluOpType

    nc = bacc.Bacc("TRN2", target_bir_lowering=False, num_swdge_queues=4)

    xb_d = nc.dram_tensor("xb", [C, L], BF, kind="ExternalInput")
    # xt pre-tiled on host to SBUF layout: xt[p, t*128+c] = x[c, t*128+p]
    xt_d = nc.dram_tensor("xt", [C, L], FP8, kind="ExternalInput")
    mrep_d = nc.dram_tensor("mrep", [C, HALF], BF, kind="ExternalInput")
    ym_d = nc.dram_tensor("ym", [C, 2], F32, kind="ExternalInput")
    out_d = nc.dram_tensor("out", [C, HALF], F32, kind="ExternalOutput")

    with tile.TileContext(nc) as tc:
        with (
            tc.tile_pool(name="big", bufs=1) as big,
            tc.tile_pool(name="small", bufs=1) as small,
            tc.tile_pool(name="sqs", bufs=2) as sqs,
            tc.tile_pool(name="work", bufs=10) as upool,
            tc.tile_pool(name="opool", bufs=4) as opool,
            tc.tile_pool(name="ps_sc", bufs=2, space=bass.MemorySpace.PSUM) as ps_sc,
            tc.tile_pool(name="ps_rec", bufs=1, space=bass.MemorySpace.PSUM) as ps_rec,
            tc.tile_pool(name="ps_sum", bufs=1, space=bass.MemorySpace.PSUM) as ps_sum,
        ):
            # ---- persistent SBUF tensors ----
            xb_sb = big.tile([C, L], BF, tag="xb_sb")      # mm1 stationary (c,l)
            xt_sb = big.tile([C, L], FP8, tag="xt_sb")     # l-major tiles (l,c)
            kn = big.tile([C, L], BF, tag="kn")            # normalized k, l-major
            y1 = big.tile([C, YW], BF, tag="y1")
            y_t = big.tile([C, HALF], BF, tag="y_t")
            mrep_sb = big.tile([C, HALF], BF, tag="mrep_sb")
            w_t = big.tile([C, HALF], F32, tag="w_t")      # (1-m)/9
            xm = big.tile([C, HALF], F32, tag="xm")        # x*m
            ones_t = small.tile([C, C], BF, tag="ones_t")
            ym_sb = small.tile([C, 2], F32, tag="ym_sb")
            norm2 = small.tile([C, NLT], F32, tag="norm2")
            rs_a = small.tile([C, NLT], F32, tag="rs_a")
            rs_b = small.tile([C, NLT], F32, tag="rs_b")
            nt_a = small.tile([C, NLT], F32, tag="nt_a")
            rs184 = small.tile([C, NLT], F32, tag="rs184")
            shift_c = small.tile([C, 1], F32, tag="shift_c")
            warm2 = small.tile([C, 1], F32, tag="warm2")

            # ---- input DMAs: only SP (sync) and ACT (scalar) have HW DMA
            # queues (~78GB/s each); order chunks by when they are needed.
            nc.sync.dma_start(xb_sb[:, 0:704], xb_d[:, 0:704])
            nc.scalar.dma_start(ym_sb[:], ym_d[:])
            nc.scalar.dma_start(xt_sb[:, 0:1024], xt_d[:, 0:1024])
            nc.sync.dma_start(xb_sb[:, 704:1280], xb_d[:, 704:1280])
            nc.scalar.dma_start(xt_sb[:, 1024:2048], xt_d[:, 1024:2048])
            nc.sync.dma_start(xb_sb[:, 1280:2176], xb_d[:, 1280:2176])
            nc.scalar.dma_start(xt_sb[:, 2048:L], xt_d[:, 2048:L])
            nc.sync.dma_start(xb_sb[:, 2176:3072], xb_d[:, 2176:3072])
            nc.sync.dma_start(xb_sb[:, 3072:L], xb_d[:, 3072:L])
            nc.scalar.dma_start(mrep_sb[:], mrep_d[:])

            # ---- tiny prologue constants ----
            nc.vector.memset(ones_t[:], 1.0)
            nc.vector.memset(shift_c[:], -SHIFT)

            # norm2[l] = sum_c xt[l, c]^2. ACT (Square+accum) for the first 8
            # tiles during the DMA head (frees the DVE head chain; costs one
            # extra Exp table reload paid before the exp stream starts); DVE
            # stt for the rest.
            def sq_chunk_act(l0, l1):
                for lt in range(l0, l1):
                    scr = sqs.tile([C, C], BF, tag="sq_scratch")
                    nc.scalar.activation(
                        scr[:], xt_sb[:, lt * C:(lt + 1) * C], AF.Square,
                        accum_out=norm2[:, lt:lt + 1])

            def sq_chunk(l0, l1):
                for lt in range(l0, l1):
                    scr = sqs.tile([C, C], BF, tag="sq_scratch")
                    nc.vector.scalar_tensor_tensor(
                        scr[:], xt_sb[:, lt * C:(lt + 1) * C], 1.0,
                        xt_sb[:, lt * C:(lt + 1) * C],
                        op0=ALU.mult, op1=ALU.mult,
                        accum_out=norm2[:, lt:lt + 1])

            # rsqrt via bit-trick seed + 2 Newton iterations (DVE, f32)
            rs_fin = rs_a

            def newton_chunk(l0, l1):
                cl = slice(l0, l1)
                nc.vector.tensor_scalar(nt_a[:, cl].bitcast(I32),
                                        norm2[:, cl].bitcast(I32), 1, None,
                                        op0=ALU.logical_shift_right)
                nc.vector.tensor_scalar(rs_a[:, cl].bitcast(I32),
                                        nt_a[:, cl].bitcast(I32),
                                        -1, 0x5f3759df,
                                        op0=ALU.mult, op1=ALU.add)
                src, dst = rs_a, rs_b
                for _ in range(2):
                    nc.vector.tensor_mul(nt_a[:, cl], src[:, cl], src[:, cl])
                    nc.vector.tensor_mul(nt_a[:, cl], nt_a[:, cl], norm2[:, cl])
                    nc.vector.tensor_scalar(nt_a[:, cl], nt_a[:, cl], -0.5, 1.5,
                                            op0=ALU.mult, op1=ALU.add)
                    nc.vector.tensor_mul(dst[:, cl], src[:, cl], nt_a[:, cl])
                    src, dst = dst, src
                nc.vector.tensor_scalar_mul(rs184[:, cl], rs_fin[:, cl],
                                            LN2_INV_128)

            def kn_chunk(l0, l1, eng):
                for lt in range(l0, l1):
                    eng.tensor_scalar_mul(
                        kn[:, lt * C:(lt + 1) * C], xt_sb[:, lt * C:(lt + 1) * C],
                        rs_fin[:, lt:lt + 1])

            # y = 3x3 box filter (row filter on xb cols [0:2176) -> y1, then
            # col filter over rotated-row positions; wrapped pad rows zeroed
            # by ym masks: pos 0 (maskA=0 iff h==0), pos 33 (maskB=0 iff h==1)
            xv = xb_sb[:, 0:YW].rearrange("p (r j) -> p r j", j=64)
            yv = y1[:].rearrange("p (r j) -> p r j", j=64)

            # --- critical-path-ordered prologue emission ---
            # ACT: squares for lt 0..8, Exp warm-up, then kn 0..8 via
            # Copy+scale (no activation table involved)
            sq_chunk_act(0, 8)
            nc.scalar.activation(warm2[:], shift_c[:], AF.Exp)
            # DVE: y part A: positions 0..9 -> y_t[0:512]
            nc.vector.tensor_add(y1[:, 1:639], xb_sb[:, 0:638],
                                 xb_sb[:, 1:639])
            nc.vector.tensor_add(y1[:, 1:639], y1[:, 1:639],
                                 xb_sb[:, 2:640])
            nc.vector.tensor_add(yv[:, 0:10, 0:1], xv[:, 0:10, 0:1],
                                 xv[:, 0:10, 1:2])
            nc.vector.tensor_add(yv[:, 0:10, 63:64], xv[:, 0:10, 62:63],
                                 xv[:, 0:10, 63:64])
            nc.vector.tensor_scalar_mul(y1[:, 0:64], y1[:, 0:64],
                                        ym_sb[:, 0:1])
            nc.vector.tensor_add(y_t[:, 0:512], y1[:, 0:512],
                                 y1[:, 64:64 + 512])
            nc.vector.tensor_add(y_t[:, 0:512], y_t[:, 0:512],
                                 y1[:, 128:128 + 512])
            # DVE: newton for lt 0..8 (norm2 from ACT squares) so the first
            # exps have their scale as soon as sc lands
            newton_chunk(0, 8)
            # DVE: y part B: positions 10..18 (interior flats [641:1216))
            nc.vector.tensor_add(y1[:, 641:1216], xb_sb[:, 640:1215],
                                 xb_sb[:, 641:1216])
            nc.vector.tensor_add(y1[:, 641:1216], y1[:, 641:1216],
                                 xb_sb[:, 642:1217])
            nc.vector.tensor_add(yv[:, 10:19, 0:1], xv[:, 10:19, 0:1],
                                 xv[:, 10:19, 1:2])
            nc.vector.tensor_add(yv[:, 10:19, 63:64], xv[:, 10:19, 62:63],
                                 xv[:, 10:19, 63:64])
            nc.vector.tensor_add(y_t[:, 512:BLK], y1[:, 512:BLK],
                                 y1[:, 512 + 64:64 + BLK])
            nc.vector.tensor_add(y_t[:, 512:BLK], y_t[:, 512:BLK],
                                 y1[:, 512 + 128:128 + BLK])
            kn_chunk(0, 8, nc.vector)

            def emit_exp(u, sc, lt, eng):
                if eng == "A":
                    nc.scalar.activation(u[:], sc[:], AF.Exp,
                                         bias=shift_c[:],
                                         scale=rs_fin[:, lt:lt + 1])
                else:
                    nc.vector.tensor_scalar(u[:].bitcast(I16), sc[:],
                                            rs184[:, lt:lt + 1], SCHR_OFF,
                                            op0=ALU.mult, op1=ALU.add)

            # ---- main loop: one global software pipeline over g = blk*32+lt
            # PE stream per slot: mm1_{g+1} then mm2_{g-1}; the exp for g runs
            # concurrently, so the ACT exp stream never waits on mm1 and the
            # DVE-assigned tiles overlap ACT tiles instead of stalling them.
            # sums tree per block: 8 groups of 4 lt tiles; groups 0..6 -> two
            # pair adds (DVE) + one L2 add (GPSIMD, off the u path) -> ones,
            # batched in pairs; group 7 -> pair sums emitted directly.
            N_ONES = 9
            NG = 2 * NLT
            st = {}   # per-block state

            def emit_mm1(g):
                blk, lt = divmod(g, NLT)
                sc = ps_sc.tile([C, BLK], F32, tag="sc", name="sc")
                for h2 in range(2):
                    cs = blk * BLK + h2 * 512
                    nc.tensor.matmul(
                        sc[:, h2 * 512:(h2 + 1) * 512],
                        xb_sb[:, lt * C:(lt + 1) * C],
                        y_t[:, cs:cs + 512],
                        start=True, stop=True,
                    )
                return sc

            def emit_mm2(g):
                blk, lt = divmod(g, NLT)
                s = st[blk]
                u = s["u"][lt]
                for h2 in range(2):
                    nc.tensor.matmul(
                        s["rec"][h2][:], kn[:, lt * C:(lt + 1) * C],
                        u[:, h2 * 512:(h2 + 1) * 512],
                        start=(lt == 0), stop=(lt == NLT - 1),
                    )

            def emit_ones(blk, w):
                s = st[blk]
                for h2 in range(2):
                    nc.tensor.matmul(
                        s["sums"][h2][:],
                        ones_t[:],
                        w[:, h2 * 512:(h2 + 1) * 512],
                        start=(s["oi"] == 0), stop=(s["oi"] == N_ONES - 1),
                    )
                s["oi"] += 1

            def emit_epilogue(blk, nchunk):
                s = st[blk]
                csz = BLK // nchunk
                for ch in range(nchunk):
                    cs = blk * BLK + ch * csz
                    h2 = (ch * csz) // 512
                    o2 = slice(ch * csz - h2 * 512, (ch + 1) * csz - h2 * 512)
                    R = opool.tile([C, csz], F32, tag=f"R{csz}", name="R")
                    nc.vector.reciprocal_approx_fast(R[:], s["sums"][h2][:, o2])
                    Rm = opool.tile([C, csz], F32, tag=f"Rm{csz}", name="Rm")
                    nc.vector.tensor_mul(Rm[:], R[:], w_t[:, cs:cs + csz])
                    ob = opool.tile([C, csz], F32, tag=f"ob{csz}", name="ob")
                    nc.vector.tensor_mul(ob[:], s["rec"][h2][:, o2], Rm[:])
                    nc.vector.tensor_add(ob[:], ob[:], xm[:, cs:cs + csz])
                    eng = nc.sync if ch % 2 == 0 else nc.scalar
                    eng.dma_start(out_d[:, cs:cs + csz], ob[:])

            sc_q = {0: emit_mm1(0)}
            for g in range(NG):
                blk, lt = divmod(g, NLT)
                if lt == 0:
                    st[blk] = {
                        "rec": [ps_rec.tile([C, 512], F32, tag=f"rec{h}",
                                            name=f"rec{h}") for h in range(2)],
                        "sums": [ps_sum.tile([C, 512], F32, tag=f"sums{h}",
                                             name=f"sums{h}") for h in range(2)],
                        "oi": 0, "wq": [], "pair": {}, "u": {},
                    }
                # block-0 prologue interleaves, paced by DMA-chunk landings;
                # wait guards keep the scheduler's model from slotting them
                # ahead of the critical first newton/kn chain
                if g == 2:
                    sq_chunk(8, 16)
                elif g == 4:
                    newton_chunk(8, 16)
                elif g == 5:
                    kn_chunk(8, 16, nc.vector)
                elif g == 7:
                    # y part C: positions 19..33 -> y_t block 1. The wait
                    # guard keeps the scheduler from slotting these long ops
                    # (whose xb chunk lands late) ahead of the newton/kn
                    # chain the first exps depend on.
                    with tc.tile_wait_until(0.024):
                        nc.vector.tensor_add(y1[:, 1216:YW - 1],
                                             xb_sb[:, 1215:YW - 2],
                                             xb_sb[:, 1216:YW - 1])
                        nc.vector.tensor_add(y1[:, 1216:YW - 1],
                                             y1[:, 1216:YW - 1],
                                             xb_sb[:, 1217:YW])
                        nc.vector.tensor_add(yv[:, 19:34, 0:1],
                                             xv[:, 19:34, 0:1],
                                             xv[:, 19:34, 1:2])
                        nc.vector.tensor_add(yv[:, 19:34, 63:64],
                                             xv[:, 19:34, 62:63],
                                             xv[:, 19:34, 63:64])
                        nc.vector.tensor_scalar_mul(y1[:, 2112:YW],
                                                    y1[:, 2112:YW],
                                                    ym_sb[:, 1:2])
                        nc.vector.tensor_add(y_t[:, BLK:HALF],
                                             y1[:, BLK:BLK + BLK],
                                             y1[:, BLK + 64:BLK + 64 + BLK])
                        nc.vector.tensor_add(y_t[:, BLK:HALF],
                                             y_t[:, BLK:HALF],
                                             y1[:, BLK + 128:BLK + 128 + BLK])
                elif g == 8:
                    sq_chunk(16, 24)
                elif g == 10:
                    newton_chunk(16, 24)
                elif g == 11:
                    kn_chunk(16, 24, nc.vector)
                elif g == 13:
                    sq_chunk(24, 32)
                elif g == 15:
                    newton_chunk(24, 32)
                elif g == 16:
                    kn_chunk(24, 32, nc.vector)
                elif g == 27:
                    # x*m / (1-m)/9 in the gap between block-0's last L2 add
                    # and block-1's first (GPSIMD is in-order; anywhere else
                    # this delays the ones chain)
                    nc.gpsimd.tensor_mul(xm[:], xb_sb[:, 64:64 + HALF],
                                         mrep_sb[:])
                elif g == 29:
                    nc.gpsimd.tensor_scalar(w_t[:], mrep_sb[:],
                                            -1.0 / 9.0, 1.0 / 9.0,
                                            op0=ALU.mult, op1=ALU.add)
                # PE: prefetch mm1 two slots ahead of the exp stream
                if g + 1 < NG:
                    sc_q[g + 1] = emit_mm1(g + 1)
                # exp for g
                sc = sc_q.pop(g)
                u = upool.tile([C, BLK], BF, tag="u", name="u")
                emit_exp(u, sc, lt, "D" if lt in DVE_TILES[blk] else "A")
                s = st[blk]
                s["u"][lt] = u
                # PE: mm2 for the previous slot
                if g >= 1:
                    emit_mm2(g - 1)
                # column-sum tree for g
                if lt % 2 == 0:
                    s["pair"]["u"] = u
                else:
                    v = upool.tile([C, BLK], BF, tag="v", name="v")
                    peng = nc.gpsimd if lt in (13, 21) else nc.vector
                    peng.tensor_add(v[:], s["pair"].pop("u")[:], u[:])
                    if lt >= NLT - 4:
                        s["wq"].append(v)     # last group: pair sums direct
                    elif lt % 4 == 1:
                        s["pair"]["v1"] = v
                    else:
                        w = upool.tile([C, BLK], BF, tag="w", name="w")
                        nc.gpsimd.tensor_add(w[:], s["pair"].pop("v1")[:], v[:])
                        s["wq"].append(w)
                        # batch ones emissions in pairs, lagged ~2 groups
                        if len(s["wq"]) > 2:
                            emit_ones(blk, s["wq"].pop(0))
                            emit_ones(blk, s["wq"].pop(0))
                # end-of-block drains ride the next block's pipeline slots
                if lt == NLT - 1:
                    if g + 1 >= NG:      # final block: drain immediately
                        emit_mm2(g)
                        for w in s["wq"]:
                            emit_ones(blk, w)
                        s["wq"] = []
                        emit_epilogue(blk, 4)
                elif lt == 0 and blk > 0:
                    pb = st[blk - 1]
                    for w in pb["wq"]:
                        emit_ones(blk - 1, w)
                    pb["wq"] = []
                elif lt == 1 and blk > 0:
                    emit_epilogue(blk - 1, 2)

    nc.finalize()
    return nc


def _get_program():
    if "nc" not in _CACHE:
        _CACHE["nc"] = _build_program()
    return _CACHE["nc"]


def _make_in_maps(fg, mk):
    in_maps = []
    for core in range(8):
        b, h = core // 2, core % 2
        start = 63 if h == 0 else 31   # rotated row order R[p] = (start+p)%64
        xi = np.roll(fg[b].reshape(C, 64, 64), -start, axis=1)
        x = np.ascontiguousarray(xi.reshape(C, L))
        xb = x.astype(NPBF16)
        # pre-tiled transpose: xt[p, t*128+c] = x[c, t*128+p]
        xt = np.ascontiguousarray(
            x.reshape(C, L // C, C).transpose(2, 1, 0).reshape(C, L)).astype(NPFP8)
        mi = np.roll(mk[b].reshape(1, 64, 64), -start, axis=1)
        mrow = mi.reshape(1, L)[:, 64:64 + HALF]
        mrep = np.ascontiguousarray(
            np.broadcast_to(mrow, (C, HALF))).astype(NPBF16)
        ym = np.empty((C, 2), np.float32)
        ym[:, 0] = 0.0 if h == 0 else 1.0
        ym[:, 1] = 0.0 if h == 1 else 1.0
        in_maps.append({"xb": xb, "xt": xt, "mrep": mrep, "ym": ym})
    return in_maps


def kernel(foreground, mask):
    fg = np.ascontiguousarray(np.asarray(foreground, dtype=np.float32))
    mk = np.ascontiguousarray(np.asarray(mask, dtype=np.float32))
    nc = _get_program()
    in_maps = _make_in_maps(fg, mk)

    from concourse.bass_utils import run_bass_kernel_spmd
    res = run_bass_kernel_spmd(nc, in_maps, core_ids=list(range(8)))

    out = np.empty((4, C, L), np.float32)
    for core in range(8):
        b, h = core // 2, core % 2
        # kernel columns = rotated positions 1..32 = image rows h*32..h*32+31
        out[b][:, h * HALF:(h + 1) * HALF] = res.results[core]["out"]
    return out.reshape(4, C, 64, 64)


# revision 37
# speedup vs baseline: 1.0642x; 1.0274x over previous
"""Trainium2 Bass kernel for CAttention (contextual attention), v2.

Math (per batch element, derived from the reference):
    x:    (c=128, h=64, w=64), flat (128, 4096); m: (1, 4096)
    k    = normalize_rows(x.reshape(c, hw).T + eps)          # (4096, 128)
    y    = 3x3 zero-padded box filter of x                   # (128, 4096)
    S    = k @ y                                             # (4096 l, 4096 ij)
    att  = softmax over l (per column); u = exp(S - 20) (S bounded, col max
           >= ~11, so a constant shift suffices; att = u / colsum(u))
    rec  = k.T @ att                                         # (128, 4096)
    out  = rec * (1-m)/9 + x*m

Sharding: pure data parallel over batch (4) x output-column halves (2) = 8
cores, zero cross-core communication.

v2 structure (vs v1): per-core row-rotated x so the y-filter slab is always
columns [0:2176) of xb (no separate xyh input; wrapped pad row zeroed via a
tiny per-core mask input); xt shipped as fp8e4 (norms + kn only); ACT engine
is exp-pure (no Square-table thrash); column-sum tree is uniform groups of 4
(pair adds on DVE, second-level adds on GPSIMD which is kept OUT of the
u-tile recycling path, last group emitted directly); kn/xm/w_t on GPSIMD.
"""

import numpy as np
import ml_dtypes

NPBF16 = ml_dtypes.bfloat16
NPFP8 = ml_dtypes.float8_e4m3fn

SHIFT = 20.0
LN2_INV_128 = 128.0 / float(np.log(2.0))   # 184.6617
SCHR_C = 5.5
SCHR_OFF = 16256.0 - SHIFT * LN2_INV_128 - SCHR_C
C = 128          # channels
L = 4096         # spatial locations (l axis)
HALF = 2048      # output columns per core
BLK = 1024       # ij block (psum-bank sized: 2 banks)
NLT = 32         # l tiles of 128
YW = 2176        # y slab width: 34 rotated image rows x 64

# exp tiles handled by DVE (i16 Schraudolph); the rest go to ACT.
DVE_TILES = [{5, 13, 21},
             {3, 11, 19, 27}]

_CACHE = {}


def _build_program():
    import concourse.bass as bass
    import concourse.bacc as bacc
    import concourse.tile as tile
    import concourse.mybir as mybir

    F32 = mybir.dt.float32
    BF = mybir.dt.bfloat16
    FP8 = mybir.dt.float8e4
    I16 = mybir.dt.int16
    I32 = mybir.dt.int32
    AF = mybir.ActivationFunctionType
    ALU = mybir.AluOpType

    nc = bacc.Bacc("TRN2", target_bir_lowering=False, num_swdge_queues=4)

    xb_d = nc.dram_tensor("xb", [C, L], BF, kind="ExternalInput")
    # xt pre-tiled on host to SBUF layout: xt[p, t*128+c] = x[c, t*128+p]
    xt_d = nc.dram_tensor("xt", [C, L], FP8, kind="ExternalInput")
    mrep_d = nc.dram_tensor("mrep", [C, HALF], BF, kind="ExternalInput")
    ym_d = nc.dram_tensor("ym", [C, 2], F32, kind="ExternalInput")
    out_d = nc.dram_tensor("out", [C, HALF], F32, kind="ExternalOutput")

    with tile.TileContext(nc) as tc:
        with (
            tc.tile_pool(name="big", bufs=1) as big,
            tc.tile_pool(name="small", bufs=1) as small,
            tc.tile_pool(name="sqs", bufs=2) as sqs,
            tc.tile_pool(name="upool", bufs=10) as upool,
            tc.tile_pool(name="vpool", bufs=6) as vpool,
            tc.tile_pool(name="wpool", bufs=4) as wpool,
            tc.tile_pool(name="opool", bufs=4) as opool,
            tc.tile_pool(name="ps_sc", bufs=2, space=bass.MemorySpace.PSUM) as ps_sc,
            tc.tile_pool(name="ps_rec", bufs=1, space=bass.MemorySpace.PSUM) as ps_rec,
            tc.tile_pool(name="ps_sum", bufs=1, space=bass.MemorySpace.PSUM) as ps_sum,
        ):
            # ---- persistent SBUF tensors ----
            xb_sb = big.tile([C, L], BF, tag="xb_sb")      # mm1 stationary (c,l)
            xt_sb = big.tile([C, L], FP8, tag="xt_sb")     # l-major tiles (l,c)
            kn = big.tile([C, L], BF, tag="kn")            # normalized k, l-major
            y1 = big.tile([C, YW], BF, tag="y1")
            y_t = big.tile([C, HALF], BF, tag="y_t")
            mrep_sb = big.tile([C, HALF], BF, tag="mrep_sb")
            w_t = big.tile([C, HALF], F32, tag="w_t")      # (1-m)/9
            xm = big.tile([C, HALF], F32, tag="xm")        # x*m
            ones_t = small.tile([C, C], BF, tag="ones_t")
            ym_sb = small.tile([C, 2], F32, tag="ym_sb")
            norm2 = small.tile([C, NLT], F32, tag="norm2")
            rs_a = small.tile([C, NLT], F32, tag="rs_a")
            rs_b = small.tile([C, NLT], F32, tag="rs_b")
            nt_a = small.tile([C, NLT], F32, tag="nt_a")
            rs184 = small.tile([C, NLT], F32, tag="rs184")
            shift_c = small.tile([C, 1], F32, tag="shift_c")
            warm2 = small.tile([C, 1], F32, tag="warm2")

            # ---- input DMAs: only SP (sync) and ACT (scalar) have HW DMA
            # queues (~78GB/s each); order chunks by when they are needed.
            nc.sync.dma_start(xb_sb[:, 0:704], xb_d[:, 0:704])
            nc.scalar.dma_start(ym_sb[:], ym_d[:])
            nc.scalar.dma_start(xt_sb[:, 0:1024], xt_d[:, 0:1024])
            nc.sync.dma_start(xb_sb[:, 704:1280], xb_d[:, 704:1280])
            nc.scalar.dma_start(xt_sb[:, 1024:2048], xt_d[:, 1024:2048])
            nc.sync.dma_start(xb_sb[:, 1280:2176], xb_d[:, 1280:2176])
            nc.scalar.dma_start(xt_sb[:, 2048:L], xt_d[:, 2048:L])
            nc.sync.dma_start(xb_sb[:, 2176:3072], xb_d[:, 2176:3072])
            nc.sync.dma_start(xb_sb[:, 3072:L], xb_d[:, 3072:L])
            nc.scalar.dma_start(mrep_sb[:], mrep_d[:])

            # ---- tiny prologue constants ----
            nc.vector.memset(ones_t[:], 1.0)
            nc.vector.memset(shift_c[:], -SHIFT)

            # norm2[l] = sum_c xt[l, c]^2. ACT (Square+accum) for the first 8
            # tiles during the DMA head (frees the DVE head chain; costs one
            # extra Exp table reload paid before the exp stream starts); DVE
            # stt for the rest.
            def sq_chunk_act(l0, l1):
                for lt in range(l0, l1):
                    scr = sqs.tile([C, C], BF, tag="sq_scratch")
                    nc.scalar.activation(
                        scr[:], xt_sb[:, lt * C:(lt + 1) * C], AF.Square,
                        accum_out=norm2[:, lt:lt + 1])

            def sq_chunk(l0, l1):
                for lt in range(l0, l1):
                    scr = sqs.tile([C, C], BF, tag="sq_scratch")
                    nc.vector.scalar_tensor_tensor(
                        scr[:], xt_sb[:, lt * C:(lt + 1) * C], 1.0,
                        xt_sb[:, lt * C:(lt + 1) * C],
                        op0=ALU.mult, op1=ALU.mult,
                        accum_out=norm2[:, lt:lt + 1])

            # rsqrt via bit-trick seed + 2 Newton iterations (DVE, f32)
            rs_fin = rs_a

            def newton_chunk(l0, l1):
                cl = slice(l0, l1)
                nc.vector.tensor_scalar(nt_a[:, cl].bitcast(I32),
                                        norm2[:, cl].bitcast(I32), 1, None,
                                        op0=ALU.logical_shift_right)
                nc.vector.tensor_scalar(rs_a[:, cl].bitcast(I32),
                                        nt_a[:, cl].bitcast(I32),
                                        -1, 0x5f3759df,
                                        op0=ALU.mult, op1=ALU.add)
                src, dst = rs_a, rs_b
                for _ in range(2):
                    nc.vector.tensor_mul(nt_a[:, cl], src[:, cl], src[:, cl])
                    nc.vector.tensor_mul(nt_a[:, cl], nt_a[:, cl], norm2[:, cl])
                    nc.vector.tensor_scalar(nt_a[:, cl], nt_a[:, cl], -0.5, 1.5,
                                            op0=ALU.mult, op1=ALU.add)
                    nc.vector.tensor_mul(dst[:, cl], src[:, cl], nt_a[:, cl])
                    src, dst = dst, src
                nc.vector.tensor_scalar_mul(rs184[:, cl], rs_fin[:, cl],
                                            LN2_INV_128)

            def kn_chunk(l0, l1, eng):
                for lt in range(l0, l1):
                    eng.tensor_scalar_mul(
                        kn[:, lt * C:(lt + 1) * C], xt_sb[:, lt * C:(lt + 1) * C],
                        rs_fin[:, lt:lt + 1])

            # y = 3x3 box filter (row filter on xb cols [0:2176) -> y1, then
            # col filter over rotated-row positions; wrapped pad rows zeroed
            # by ym masks: pos 0 (maskA=0 iff h==0), pos 33 (maskB=0 iff h==1)
            xv = xb_sb[:, 0:YW].rearrange("p (r j) -> p r j", j=64)
            yv = y1[:].rearrange("p (r j) -> p r j", j=64)

            # --- critical-path-ordered prologue emission ---
            # ACT: squares for lt 0..8, Exp warm-up, then kn 0..8 via
            # Copy+scale (no activation table involved)
            sq_chunk_act(0, 8)
            nc.scalar.activation(warm2[:], shift_c[:], AF.Exp)
            # DVE: y part A: positions 0..9 -> y_t[0:512]
            nc.vector.tensor_add(y1[:, 1:639], xb_sb[:, 0:638],
                                 xb_sb[:, 1:639])
            nc.vector.tensor_add(y1[:, 1:639], y1[:, 1:639],
                                 xb_sb[:, 2:640])
            nc.vector.tensor_add(yv[:, 0:10, 0:1], xv[:, 0:10, 0:1],
                                 xv[:, 0:10, 1:2])
            nc.vector.tensor_add(yv[:, 0:10, 63:64], xv[:, 0:10, 62:63],
                                 xv[:, 0:10, 63:64])
            nc.vector.tensor_scalar_mul(y1[:, 0:64], y1[:, 0:64],
                                        ym_sb[:, 0:1])
            nc.vector.tensor_add(y_t[:, 0:512], y1[:, 0:512],
                                 y1[:, 64:64 + 512])
            nc.vector.tensor_add(y_t[:, 0:512], y_t[:, 0:512],
                                 y1[:, 128:128 + 512])
            # DVE: newton for lt 0..8 (norm2 from ACT squares) so the first
            # exps have their scale as soon as sc lands
            newton_chunk(0, 8)
            # DVE: y part B: positions 10..18 (interior flats [641:1216))
            nc.vector.tensor_add(y1[:, 641:1216], xb_sb[:, 640:1215],
                                 xb_sb[:, 641:1216])
            nc.vector.tensor_add(y1[:, 641:1216], y1[:, 641:1216],
                                 xb_sb[:, 642:1217])
            nc.vector.tensor_add(yv[:, 10:19, 0:1], xv[:, 10:19, 0:1],
                                 xv[:, 10:19, 1:2])
            nc.vector.tensor_add(yv[:, 10:19, 63:64], xv[:, 10:19, 62:63],
                                 xv[:, 10:19, 63:64])
            nc.vector.tensor_add(y_t[:, 512:BLK], y1[:, 512:BLK],
                                 y1[:, 512 + 64:64 + BLK])
            nc.vector.tensor_add(y_t[:, 512:BLK], y_t[:, 512:BLK],
                                 y1[:, 512 + 128:128 + BLK])
            kn_chunk(0, 8, nc.vector)

            def emit_exp(u, sc, lt, eng):
                if eng == "A":
                    nc.scalar.activation(u[:], sc[:], AF.Exp,
                                         bias=shift_c[:],
                                         scale=rs_fin[:, lt:lt + 1])
                else:
                    nc.vector.tensor_scalar(u[:].bitcast(I16), sc[:],
                                            rs184[:, lt:lt + 1], SCHR_OFF,
                                            op0=ALU.mult, op1=ALU.add)

            # ---- main loop: one global software pipeline over g = blk*32+lt
            # PE stream per slot: mm1_{g+1} then mm2_{g-1}; the exp for g runs
            # concurrently, so the ACT exp stream never waits on mm1 and the
            # DVE-assigned tiles overlap ACT tiles instead of stalling them.
            # sums tree per block: 8 groups of 4 lt tiles; groups 0..6 -> two
            # pair adds (DVE) + one L2 add (GPSIMD, off the u path) -> ones,
            # batched in pairs; group 7 -> pair sums emitted directly.
            N_ONES = 9
            NG = 2 * NLT
            st = {}   # per-block state

            def emit_mm1(g):
                blk, lt = divmod(g, NLT)
                sc = ps_sc.tile([C, BLK], F32, tag="sc", name="sc")
                for h2 in range(2):
                    cs = blk * BLK + h2 * 512
                    nc.tensor.matmul(
                        sc[:, h2 * 512:(h2 + 1) * 512],
                        xb_sb[:, lt * C:(lt + 1) * C],
                        y_t[:, cs:cs + 512],
                        start=True, stop=True,
                    )
                return sc

            def emit_mm2(g):
                blk, lt = divmod(g, NLT)
                s = st[blk]
                u = s["u"][lt]
                for h2 in range(2):
                    nc.tensor.matmul(
                        s["rec"][h2][:], kn[:, lt * C:(lt + 1) * C],
                        u[:, h2 * 512:(h2 + 1) * 512],
                        start=(lt == 0), stop=(lt == NLT - 1),
                    )

            def emit_ones(blk, w):
                s = st[blk]
                for h2 in range(2):
                    nc.tensor.matmul(
                        s["sums"][h2][:],
                        ones_t[:],
                        w[:, h2 * 512:(h2 + 1) * 512],
                        start=(s["oi"] == 0), stop=(s["oi"] == N_ONES - 1),
                    )
                s["oi"] += 1

            def emit_epilogue(blk, nchunk):
                s = st[blk]
                csz = BLK // nchunk
                for ch in range(nchunk):
                    cs = blk * BLK + ch * csz
                    h2 = (ch * csz) // 512
                    o2 = slice(ch * csz - h2 * 512, (ch + 1) * csz - h2 * 512)
                    R = opool.tile([C, csz], F32, tag=f"R{csz}", name="R")
                    nc.vector.reciprocal_approx_fast(R[:], s["sums"][h2][:, o2])
                    Rm = opool.tile([C, csz], F32, tag=f"Rm{csz}", name="Rm")
                    nc.vector.tensor_mul(Rm[:], R[:], w_t[:, cs:cs + csz])
                    ob = opool.tile([C, csz], F32, tag=f"ob{csz}", name="ob")
                    nc.vector.tensor_mul(ob[:], s["rec"][h2][:, o2], Rm[:])
                    nc.vector.tensor_add(ob[:], ob[:], xm[:, cs:cs + csz])
                    eng = nc.sync if ch % 2 == 0 else nc.scalar
                    eng.dma_start(out_d[:, cs:cs + csz], ob[:])

            sc_q = {0: emit_mm1(0)}
            for g in range(NG):
                blk, lt = divmod(g, NLT)
                if lt == 0:
                    st[blk] = {
                        "rec": [ps_rec.tile([C, 512], F32, tag=f"rec{h}",
                                            name=f"rec{h}") for h in range(2)],
                        "sums": [ps_sum.tile([C, 512], F32, tag=f"sums{h}",
                                             name=f"sums{h}") for h in range(2)],
                        "oi": 0, "wq": [], "pair": {}, "u": {},
                    }
                # block-0 prologue interleaves, paced by DMA-chunk landings;
                # wait guards keep the scheduler's model from slotting them
                # ahead of the critical first newton/kn chain
                if g == 2:
                    sq_chunk(8, 16)
                elif g == 4:
                    newton_chunk(8, 16)
                elif g == 5:
                    kn_chunk(8, 16, nc.vector)
                elif g == 7:
                    # y part C: positions 19..33 -> y_t block 1. The wait
                    # guard keeps the scheduler from slotting these long ops
                    # (whose xb chunk lands late) ahead of the newton/kn
                    # chain the first exps depend on.
                    with tc.tile_wait_until(0.024):
                        nc.vector.tensor_add(y1[:, 1216:YW - 1],
                                             xb_sb[:, 1215:YW - 2],
                                             xb_sb[:, 1216:YW - 1])
                        nc.vector.tensor_add(y1[:, 1216:YW - 1],
                                             y1[:, 1216:YW - 1],
                                             xb_sb[:, 1217:YW])
                        nc.vector.tensor_add(yv[:, 19:34, 0:1],
                                             xv[:, 19:34, 0:1],
                                             xv[:, 19:34, 1:2])
                        nc.vector.tensor_add(yv[:, 19:34, 63:64],
                                             xv[:, 19:34, 62:63],
                                             xv[:, 19:34, 63:64])
                        nc.vector.tensor_scalar_mul(y1[:, 2112:YW],
                                                    y1[:, 2112:YW],
                                                    ym_sb[:, 1:2])
                        nc.vector.tensor_add(y_t[:, BLK:HALF],
                                             y1[:, BLK:BLK + BLK],
                                             y1[:, BLK + 64:BLK + 64 + BLK])
                        nc.vector.tensor_add(y_t[:, BLK:HALF],
                                             y_t[:, BLK:HALF],
                                             y1[:, BLK + 128:BLK + 128 + BLK])
                elif g == 8:
                    sq_chunk(16, 24)
                elif g == 10:
                    newton_chunk(16, 24)
                elif g == 11:
                    kn_chunk(16, 24, nc.vector)
                elif g == 13:
                    sq_chunk(24, 32)
                elif g == 15:
                    newton_chunk(24, 32)
                elif g == 16:
                    kn_chunk(24, 32, nc.vector)
                elif g == 27:
                    # x*m / (1-m)/9 in the gap between block-0's last L2 add
                    # and block-1's first (GPSIMD is in-order; anywhere else
                    # this delays the ones chain)
                    nc.gpsimd.tensor_mul(xm[:], xb_sb[:, 64:64 + HALF],
                                         mrep_sb[:])
                elif g == 29:
                    nc.gpsimd.tensor_scalar(w_t[:], mrep_sb[:],
                                            -1.0 / 9.0, 1.0 / 9.0,
                                            op0=ALU.mult, op1=ALU.add)
                # PE: prefetch mm1 two slots ahead of the exp stream
                if g + 1 < NG:
                    sc_q[g + 1] = emit_mm1(g + 1)
                # exp for g
                sc = sc_q.pop(g)
                u = upool.tile([C, BLK], BF, tag="u", name="u")
                emit_exp(u, sc, lt, "D" if lt in DVE_TILES[blk] else "A")
                s = st[blk]
                s["u"][lt] = u
                # PE: mm2 for the previous slot
                if g >= 1:
                    emit_mm2(g - 1)
                # column-sum tree for g
                if lt % 2 == 0:
                    s["pair"]["u"] = u
                else:
                    v = vpool.tile([C, BLK], BF, tag="v", name="v")
                    nc.vector.tensor_add(v[:], s["pair"].pop("u")[:], u[:])
                    if lt >= NLT - 4:
                        s["wq"].append(v)     # last group: pair sums direct
                    elif lt % 4 == 1:
                        s["pair"]["v1"] = v
                    else:
                        w = wpool.tile([C, BLK], BF, tag="w", name="w")
                        nc.gpsimd.tensor_add(w[:], s["pair"].pop("v1")[:], v[:])
                        s["wq"].append(w)
                        # batch ones emissions in pairs, lagged ~2 groups
                        if len(s["wq"]) > 2:
                            emit_ones(blk, s["wq"].pop(0))
                            emit_ones(blk, s["wq"].pop(0))
                # end-of-block drains ride the next block's pipeline slots
                if lt == NLT - 1:
                    if g + 1 >= NG:      # final block: drain immediately
                        emit_mm2(g)
                        for w in s["wq"]:
                            emit_ones(blk, w)
                        s["wq"] = []
                        emit_epilogue(blk, 4)
                elif lt == 0 and blk > 0:
                    pb = st[blk - 1]
                    for w in pb["wq"]:
                        emit_ones(blk - 1, w)
                    pb["wq"] = []
                elif lt == 1 and blk > 0:
                    emit_epilogue(blk - 1, 2)

    nc.finalize()
    return nc


def _get_program():
    if "nc" not in _CACHE:
        _CACHE["nc"] = _build_program()
    return _CACHE["nc"]


def _make_in_maps(fg, mk):
    in_maps = []
    for core in range(8):
        b, h = core // 2, core % 2
        start = 63 if h == 0 else 31   # rotated row order R[p] = (start+p)%64
        xi = np.roll(fg[b].reshape(C, 64, 64), -start, axis=1)
        x = np.ascontiguousarray(xi.reshape(C, L))
        xb = x.astype(NPBF16)
        # pre-tiled transpose: xt[p, t*128+c] = x[c, t*128+p]
        xt = np.ascontiguousarray(
            x.reshape(C, L // C, C).transpose(2, 1, 0).reshape(C, L)).astype(NPFP8)
        mi = np.roll(mk[b].reshape(1, 64, 64), -start, axis=1)
        mrow = mi.reshape(1, L)[:, 64:64 + HALF]
        mrep = np.ascontiguousarray(
            np.broadcast_to(mrow, (C, HALF))).astype(NPBF16)
        ym = np.empty((C, 2), np.float32)
        ym[:, 0] = 0.0 if h == 0 else 1.0
        ym[:, 1] = 0.0 if h == 1 else 1.0
        in_maps.append({"xb": xb, "xt": xt, "mrep": mrep, "ym": ym})
    return in_maps


def kernel(foreground, mask):
    fg = np.ascontiguousarray(np.asarray(foreground, dtype=np.float32))
    mk = np.ascontiguousarray(np.asarray(mask, dtype=np.float32))
    nc = _get_program()
    in_maps = _make_in_maps(fg, mk)

    from concourse.bass_utils import run_bass_kernel_spmd
    res = run_bass_kernel_spmd(nc, in_maps, core_ids=list(range(8)))

    out = np.empty((4, C, L), np.float32)
    for core in range(8):
        b, h = core // 2, core % 2
        # kernel columns = rotated positions 1..32 = image rows h*32..h*32+31
        out[b][:, h * HALF:(h + 1) * HALF] = res.results[core]["out"]
    return out.reshape(4, C, 64, 64)


# revision 40
# speedup vs baseline: 1.1095x; 1.0426x over previous
"""Trainium2 Bass kernel for CAttention (contextual attention), v2.

Math (per batch element, derived from the reference):
    x:    (c=128, h=64, w=64), flat (128, 4096); m: (1, 4096)
    k    = normalize_rows(x.reshape(c, hw).T + eps)          # (4096, 128)
    y    = 3x3 zero-padded box filter of x                   # (128, 4096)
    S    = k @ y                                             # (4096 l, 4096 ij)
    att  = softmax over l (per column); u = exp(S - 20) (S bounded, col max
           >= ~11, so a constant shift suffices; att = u / colsum(u))
    rec  = k.T @ att                                         # (128, 4096)
    out  = rec * (1-m)/9 + x*m

Sharding: pure data parallel over batch (4) x output-column halves (2) = 8
cores, zero cross-core communication.

v2 structure (vs v1): per-core row-rotated x so the y-filter slab is always
columns [0:2176) of xb (no separate xyh input; wrapped pad row zeroed via a
tiny per-core mask input); xt shipped as fp8e4 (norms + kn only); ACT engine
is exp-pure (no Square-table thrash); column-sum tree is uniform groups of 4
(pair adds on DVE, second-level adds on GPSIMD which is kept OUT of the
u-tile recycling path, last group emitted directly); kn/xm/w_t on GPSIMD.
"""

import numpy as np
import ml_dtypes

NPBF16 = ml_dtypes.bfloat16
NPFP8 = ml_dtypes.float8_e4m3fn

SHIFT = 20.0
LN2_INV_128 = 128.0 / float(np.log(2.0))   # 184.6617
SCHR_C = 5.5
SCHR_OFF = 16256.0 - SHIFT * LN2_INV_128 - SCHR_C
C = 128          # channels
L = 4096         # spatial locations (l axis)
HALF = 2048      # output columns per core
BLK = 1024       # ij block (psum-bank sized: 2 banks)
NLT = 32         # l tiles of 128
YW = 2176        # y slab width: 34 rotated image rows x 64

# exp tiles handled by DVE (i16 Schraudolph); the rest go to ACT.
DVE_TILES = [{5, 13, 21},
             {3, 11, 19, 27}]

_CACHE = {}


def _build_program():
    import concourse.bass as bass
    import concourse.bacc as bacc
    import concourse.tile as tile
    import concourse.mybir as mybir

    F32 = mybir.dt.float32
    BF = mybir.dt.bfloat16
    FP8 = mybir.dt.float8e4
    I16 = mybir.dt.int16
    I32 = mybir.dt.int32
    AF = mybir.ActivationFunctionType
    ALU = mybir.AluOpType

    nc = bacc.Bacc("TRN2", target_bir_lowering=False, num_swdge_queues=4)

    xb_d = nc.dram_tensor("xb", [C, L], BF, kind="ExternalInput")
    # xt pre-tiled on host to SBUF layout: xt[p, t*128+c] = x[c, t*128+p]
    xt_d = nc.dram_tensor("xt", [C, L], FP8, kind="ExternalInput")
    mrep_d = nc.dram_tensor("mrep", [C, HALF], BF, kind="ExternalInput")
    ym_d = nc.dram_tensor("ym", [C, 2], F32, kind="ExternalInput")
    out_d = nc.dram_tensor("out", [C, HALF], F32, kind="ExternalOutput")

    with tile.TileContext(nc) as tc:
        with (
            tc.tile_pool(name="big", bufs=1) as big,
            tc.tile_pool(name="small", bufs=1) as small,
            tc.tile_pool(name="sqs", bufs=2) as sqs,
            tc.tile_pool(name="upool", bufs=10) as upool,
            tc.tile_pool(name="vpool", bufs=6) as vpool,
            tc.tile_pool(name="wpool", bufs=4) as wpool,
            tc.tile_pool(name="opool", bufs=4) as opool,
            tc.tile_pool(name="ps_sc", bufs=2, space=bass.MemorySpace.PSUM) as ps_sc,
            tc.tile_pool(name="ps_rec", bufs=1, space=bass.MemorySpace.PSUM) as ps_rec,
            tc.tile_pool(name="ps_sum", bufs=1, space=bass.MemorySpace.PSUM) as ps_sum,
        ):
            # ---- persistent SBUF tensors ----
            xb_sb = big.tile([C, L], BF, tag="xb_sb")      # mm1 stationary (c,l)
            xt_sb = big.tile([C, L], FP8, tag="xt_sb")     # l-major tiles (l,c)
            kn = big.tile([C, L], BF, tag="kn")            # normalized k, l-major
            y1 = big.tile([C, YW], BF, tag="y1")
            y_t = big.tile([C, HALF], BF, tag="y_t")
            mrep_sb = big.tile([C, HALF], BF, tag="mrep_sb")
            w_t = big.tile([C, HALF], F32, tag="w_t")      # (1-m)/9
            xm = big.tile([C, HALF], F32, tag="xm")        # x*m
            ones_t = small.tile([C, C], BF, tag="ones_t")
            ym_sb = small.tile([C, 2], F32, tag="ym_sb")
            norm2 = small.tile([C, NLT], F32, tag="norm2")
            rs_a = small.tile([C, NLT], F32, tag="rs_a")
            rs_b = small.tile([C, NLT], F32, tag="rs_b")
            nt_a = small.tile([C, NLT], F32, tag="nt_a")
            rs184 = small.tile([C, NLT], F32, tag="rs184")
            shift_c = small.tile([C, 1], F32, tag="shift_c")
            warm2 = small.tile([C, 1], F32, tag="warm2")

            # ---- input DMAs: only SP (sync) and ACT (scalar) have HW DMA
            # queues (~78GB/s each); order chunks by when they are needed.
            nc.sync.dma_start(xb_sb[:, 0:704], xb_d[:, 0:704])
            nc.scalar.dma_start(ym_sb[:], ym_d[:])
            nc.scalar.dma_start(xt_sb[:, 0:1024], xt_d[:, 0:1024])
            nc.sync.dma_start(xb_sb[:, 704:1280], xb_d[:, 704:1280])
            nc.scalar.dma_start(xt_sb[:, 1024:2048], xt_d[:, 1024:2048])
            nc.sync.dma_start(xb_sb[:, 1280:2176], xb_d[:, 1280:2176])
            nc.scalar.dma_start(xt_sb[:, 2048:L], xt_d[:, 2048:L])
            nc.sync.dma_start(xb_sb[:, 2176:3072], xb_d[:, 2176:3072])
            nc.sync.dma_start(xb_sb[:, 3072:L], xb_d[:, 3072:L])
            nc.scalar.dma_start(mrep_sb[:], mrep_d[:])

            # ---- tiny prologue constants ----
            nc.vector.memset(ones_t[:], 1.0)
            nc.vector.memset(shift_c[:], -SHIFT)

            # norm2[l] = sum_c xt[l, c]^2. ACT (Square+accum) for the first 8
            # tiles during the DMA head (frees the DVE head chain; costs one
            # extra Exp table reload paid before the exp stream starts); DVE
            # stt for the rest.
            def sq_chunk_act(l0, l1):
                for lt in range(l0, l1):
                    scr = sqs.tile([C, C], BF, tag="sq_scratch")
                    nc.scalar.activation(
                        scr[:], xt_sb[:, lt * C:(lt + 1) * C], AF.Square,
                        accum_out=norm2[:, lt:lt + 1])

            def sq_chunk(l0, l1):
                for lt in range(l0, l1):
                    scr = sqs.tile([C, C], BF, tag="sq_scratch")
                    nc.vector.scalar_tensor_tensor(
                        scr[:], xt_sb[:, lt * C:(lt + 1) * C], 1.0,
                        xt_sb[:, lt * C:(lt + 1) * C],
                        op0=ALU.mult, op1=ALU.mult,
                        accum_out=norm2[:, lt:lt + 1])

            # rsqrt via bit-trick seed + 2 Newton iterations (DVE, f32)
            rs_fin = rs_a

            def newton_chunk(l0, l1):
                cl = slice(l0, l1)
                nc.vector.tensor_scalar(nt_a[:, cl].bitcast(I32),
                                        norm2[:, cl].bitcast(I32), 1, None,
                                        op0=ALU.logical_shift_right)
                nc.vector.tensor_scalar(rs_a[:, cl].bitcast(I32),
                                        nt_a[:, cl].bitcast(I32),
                                        -1, 0x5f3759df,
                                        op0=ALU.mult, op1=ALU.add)
                src, dst = rs_a, rs_b
                for _ in range(2):
                    nc.vector.tensor_mul(nt_a[:, cl], src[:, cl], src[:, cl])
                    nc.vector.tensor_mul(nt_a[:, cl], nt_a[:, cl], norm2[:, cl])
                    nc.vector.tensor_scalar(nt_a[:, cl], nt_a[:, cl], -0.5, 1.5,
                                            op0=ALU.mult, op1=ALU.add)
                    nc.vector.tensor_mul(dst[:, cl], src[:, cl], nt_a[:, cl])
                    src, dst = dst, src
                nc.vector.tensor_scalar_mul(rs184[:, cl], rs_fin[:, cl],
                                            LN2_INV_128)

            def kn_chunk(l0, l1, eng):
                for lt in range(l0, l1):
                    eng.tensor_scalar_mul(
                        kn[:, lt * C:(lt + 1) * C], xt_sb[:, lt * C:(lt + 1) * C],
                        rs_fin[:, lt:lt + 1])

            # y = 3x3 box filter (row filter on xb cols [0:2176) -> y1, then
            # col filter over rotated-row positions; wrapped pad rows zeroed
            # by ym masks: pos 0 (maskA=0 iff h==0), pos 33 (maskB=0 iff h==1)
            xv = xb_sb[:, 0:YW].rearrange("p (r j) -> p r j", j=64)
            yv = y1[:].rearrange("p (r j) -> p r j", j=64)

            # --- critical-path-ordered prologue emission ---
            # ACT: squares for lt 0..8, Exp warm-up, then kn 0..8 via
            # Copy+scale (no activation table involved)
            sq_chunk_act(0, 8)
            nc.scalar.activation(warm2[:], shift_c[:], AF.Exp)
            # DVE: y part A: positions 0..9 -> y_t[0:512]
            nc.vector.tensor_add(y1[:, 1:639], xb_sb[:, 0:638],
                                 xb_sb[:, 1:639])
            nc.vector.tensor_add(y1[:, 1:639], y1[:, 1:639],
                                 xb_sb[:, 2:640])
            nc.vector.tensor_add(yv[:, 0:10, 0:1], xv[:, 0:10, 0:1],
                                 xv[:, 0:10, 1:2])
            nc.vector.tensor_add(yv[:, 0:10, 63:64], xv[:, 0:10, 62:63],
                                 xv[:, 0:10, 63:64])
            nc.vector.tensor_scalar_mul(y1[:, 0:64], y1[:, 0:64],
                                        ym_sb[:, 0:1])
            nc.vector.tensor_add(y_t[:, 0:512], y1[:, 0:512],
                                 y1[:, 64:64 + 512])
            nc.vector.tensor_add(y_t[:, 0:512], y_t[:, 0:512],
                                 y1[:, 128:128 + 512])
            # DVE: newton for lt 0..8 (norm2 from ACT squares) so the first
            # exps have their scale as soon as sc lands
            newton_chunk(0, 8)
            # DVE: y part B: positions 10..18 (interior flats [641:1216))
            nc.vector.tensor_add(y1[:, 641:1216], xb_sb[:, 640:1215],
                                 xb_sb[:, 641:1216])
            nc.vector.tensor_add(y1[:, 641:1216], y1[:, 641:1216],
                                 xb_sb[:, 642:1217])
            nc.vector.tensor_add(yv[:, 10:19, 0:1], xv[:, 10:19, 0:1],
                                 xv[:, 10:19, 1:2])
            nc.vector.tensor_add(yv[:, 10:19, 63:64], xv[:, 10:19, 62:63],
                                 xv[:, 10:19, 63:64])
            nc.vector.tensor_add(y_t[:, 512:BLK], y1[:, 512:BLK],
                                 y1[:, 512 + 64:64 + BLK])
            nc.vector.tensor_add(y_t[:, 512:BLK], y_t[:, 512:BLK],
                                 y1[:, 512 + 128:128 + BLK])
            kn_chunk(0, 8, nc.vector)

            def emit_exp(u, sc, lt, eng):
                if eng == "A":
                    nc.scalar.activation(u[:], sc[:], AF.Exp,
                                         bias=shift_c[:],
                                         scale=rs_fin[:, lt:lt + 1])
                else:
                    nc.vector.tensor_scalar(u[:].bitcast(I16), sc[:],
                                            rs184[:, lt:lt + 1], SCHR_OFF,
                                            op0=ALU.mult, op1=ALU.add)

            # ---- main loop: one global software pipeline over g = blk*32+lt
            # PE stream per slot: mm1_{g+1} then mm2_{g-1}; the exp for g runs
            # concurrently, so the ACT exp stream never waits on mm1 and the
            # DVE-assigned tiles overlap ACT tiles instead of stalling them.
            # sums tree per block: 8 groups of 4 lt tiles; groups 0..6 -> two
            # pair adds (DVE) + one L2 add (GPSIMD, off the u path) -> ones,
            # batched in pairs; group 7 -> pair sums emitted directly.
            N_ONES = 9
            NG = 2 * NLT
            st = {}   # per-block state

            def emit_mm1(g):
                blk, lt = divmod(g, NLT)
                sc = ps_sc.tile([C, BLK], F32, tag="sc", name="sc")
                for h2 in range(2):
                    cs = blk * BLK + h2 * 512
                    nc.tensor.matmul(
                        sc[:, h2 * 512:(h2 + 1) * 512],
                        xb_sb[:, lt * C:(lt + 1) * C],
                        y_t[:, cs:cs + 512],
                        start=True, stop=True,
                    )
                return sc

            def emit_mm2(g):
                blk, lt = divmod(g, NLT)
                s = st[blk]
                u = s["u"][lt]
                for h2 in range(2):
                    nc.tensor.matmul(
                        s["rec"][h2][:], kn[:, lt * C:(lt + 1) * C],
                        u[:, h2 * 512:(h2 + 1) * 512],
                        start=(lt == 0), stop=(lt == NLT - 1),
                    )

            def emit_ones(blk, w):
                s = st[blk]
                for h2 in range(2):
                    nc.tensor.matmul(
                        s["sums"][h2][:],
                        ones_t[:],
                        w[:, h2 * 512:(h2 + 1) * 512],
                        start=(s["oi"] == 0), stop=(s["oi"] == N_ONES - 1),
                    )
                s["oi"] += 1

            def emit_epilogue(blk, nchunk):
                s = st[blk]
                csz = BLK // nchunk
                for ch in range(nchunk):
                    cs = blk * BLK + ch * csz
                    h2 = (ch * csz) // 512
                    o2 = slice(ch * csz - h2 * 512, (ch + 1) * csz - h2 * 512)
                    R = opool.tile([C, csz], F32, tag=f"R{csz}", name="R")
                    nc.vector.reciprocal_approx_fast(R[:], s["sums"][h2][:, o2])
                    Rm = opool.tile([C, csz], F32, tag=f"Rm{csz}", name="Rm")
                    nc.vector.tensor_mul(Rm[:], R[:], w_t[:, cs:cs + csz])
                    ob = opool.tile([C, csz], F32, tag=f"ob{csz}", name="ob")
                    nc.vector.tensor_mul(ob[:], s["rec"][h2][:, o2], Rm[:])
                    nc.vector.tensor_add(ob[:], ob[:], xm[:, cs:cs + csz])
                    eng = nc.sync if ch % 2 == 0 else nc.scalar
                    eng.dma_start(out_d[:, cs:cs + csz], ob[:])

            sc_q = {0: emit_mm1(0)}
            for g in range(NG):
                blk, lt = divmod(g, NLT)
                if lt == 0:
                    st[blk] = {
                        "rec": [ps_rec.tile([C, 512], F32, tag=f"rec{h}",
                                            name=f"rec{h}") for h in range(2)],
                        "sums": [ps_sum.tile([C, 512], F32, tag=f"sums{h}",
                                             name=f"sums{h}") for h in range(2)],
                        "oi": 0, "wq": [], "pair": {}, "u": {},
                    }
                # block-0 prologue interleaves, paced by DMA-chunk landings;
                # wait guards keep the scheduler's model from slotting them
                # ahead of the critical first newton/kn chain
                if g == 6:
                    sq_chunk(8, 16)
                elif g == 8:
                    newton_chunk(8, 16)
                elif g == 9:
                    kn_chunk(8, 16, nc.vector)
                elif g == 7:
                    # y part C: positions 19..33 -> y_t block 1. The wait
                    # guard keeps the scheduler from slotting these long ops
                    # (whose xb chunk lands late) ahead of the newton/kn
                    # chain the first exps depend on.
                    with tc.tile_wait_until(0.024):
                        nc.vector.tensor_add(y1[:, 1216:YW - 1],
                                             xb_sb[:, 1215:YW - 2],
                                             xb_sb[:, 1216:YW - 1])
                        nc.vector.tensor_add(y1[:, 1216:YW - 1],
                                             y1[:, 1216:YW - 1],
                                             xb_sb[:, 1217:YW])
                        nc.vector.tensor_add(yv[:, 19:34, 0:1],
                                             xv[:, 19:34, 0:1],
                                             xv[:, 19:34, 1:2])
                        nc.vector.tensor_add(yv[:, 19:34, 63:64],
                                             xv[:, 19:34, 62:63],
                                             xv[:, 19:34, 63:64])
                        nc.vector.tensor_scalar_mul(y1[:, 2112:YW],
                                                    y1[:, 2112:YW],
                                                    ym_sb[:, 1:2])
                        nc.vector.tensor_add(y_t[:, BLK:HALF],
                                             y1[:, BLK:BLK + BLK],
                                             y1[:, BLK + 64:BLK + 64 + BLK])
                        nc.vector.tensor_add(y_t[:, BLK:HALF],
                                             y_t[:, BLK:HALF],
                                             y1[:, BLK + 128:BLK + 128 + BLK])
                elif g == 12:
                    sq_chunk(16, 24)
                elif g == 14:
                    newton_chunk(16, 24)
                elif g == 15:
                    kn_chunk(16, 24, nc.vector)
                elif g == 18:
                    sq_chunk(24, 32)
                elif g == 20:
                    newton_chunk(24, 32)
                elif g == 21:
                    kn_chunk(24, 32, nc.vector)
                elif g == 27:
                    # x*m / (1-m)/9 in the gap between block-0's last L2 add
                    # and block-1's first (GPSIMD is in-order; anywhere else
                    # this delays the ones chain)
                    nc.gpsimd.tensor_mul(xm[:], xb_sb[:, 64:64 + HALF],
                                         mrep_sb[:])
                elif g == 29:
                    nc.gpsimd.tensor_scalar(w_t[:], mrep_sb[:],
                                            -1.0 / 9.0, 1.0 / 9.0,
                                            op0=ALU.mult, op1=ALU.add)
                # PE: prefetch mm1 two slots ahead of the exp stream
                if g + 1 < NG:
                    sc_q[g + 1] = emit_mm1(g + 1)
                # exp for g
                sc = sc_q.pop(g)
                u = upool.tile([C, BLK], BF, tag="u", name="u")
                emit_exp(u, sc, lt, "D" if lt in DVE_TILES[blk] else "A")
                s = st[blk]
                s["u"][lt] = u
                # PE: mm2 for the previous slot
                if g >= 1:
                    emit_mm2(g - 1)
                # column-sum tree for g
                if lt % 2 == 0:
                    s["pair"]["u"] = u
                else:
                    v = vpool.tile([C, BLK], BF, tag="v", name="v")
                    nc.vector.tensor_add(v[:], s["pair"].pop("u")[:], u[:])
                    if lt >= NLT - 4:
                        s["wq"].append(v)     # last group: pair sums direct
                    elif lt % 4 == 1:
                        s["pair"]["v1"] = v
                    else:
                        w = wpool.tile([C, BLK], BF, tag="w", name="w")
                        # guard: don't let GPSIMD start spinning on the v
                        # semaphores long before the exps can produce them
                        with tc.tile_wait_until(0.020 + g * 0.0011):
                            nc.gpsimd.tensor_add(w[:], s["pair"].pop("v1")[:],
                                                 v[:])
                        s["wq"].append(w)
                        # batch ones emissions in pairs, lagged ~2 groups
                        if len(s["wq"]) > 2:
                            emit_ones(blk, s["wq"].pop(0))
                            emit_ones(blk, s["wq"].pop(0))
                # end-of-block drains ride the next block's pipeline slots
                if lt == NLT - 1:
                    if g + 1 >= NG:      # final block: drain immediately
                        emit_mm2(g)
                        for w in s["wq"]:
                            emit_ones(blk, w)
                        s["wq"] = []
                        emit_epilogue(blk, 4)
                elif lt == 0 and blk > 0:
                    pb = st[blk - 1]
                    for w in pb["wq"]:
                        emit_ones(blk - 1, w)
                    pb["wq"] = []
                elif lt == 1 and blk > 0:
                    emit_epilogue(blk - 1, 2)

    nc.finalize()
    return nc


def _get_program():
    if "nc" not in _CACHE:
        _CACHE["nc"] = _build_program()
    return _CACHE["nc"]


def _make_in_maps(fg, mk):
    in_maps = []
    for core in range(8):
        b, h = core // 2, core % 2
        start = 63 if h == 0 else 31   # rotated row order R[p] = (start+p)%64
        xi = np.roll(fg[b].reshape(C, 64, 64), -start, axis=1)
        x = np.ascontiguousarray(xi.reshape(C, L))
        xb = x.astype(NPBF16)
        # pre-tiled transpose: xt[p, t*128+c] = x[c, t*128+p]
        xt = np.ascontiguousarray(
            x.reshape(C, L // C, C).transpose(2, 1, 0).reshape(C, L)).astype(NPFP8)
        mi = np.roll(mk[b].reshape(1, 64, 64), -start, axis=1)
        mrow = mi.reshape(1, L)[:, 64:64 + HALF]
        mrep = np.ascontiguousarray(
            np.broadcast_to(mrow, (C, HALF))).astype(NPBF16)
        ym = np.empty((C, 2), np.float32)
        ym[:, 0] = 0.0 if h == 0 else 1.0
        ym[:, 1] = 0.0 if h == 1 else 1.0
        in_maps.append({"xb": xb, "xt": xt, "mrep": mrep, "ym": ym})
    return in_maps


def kernel(foreground, mask):
    fg = np.ascontiguousarray(np.asarray(foreground, dtype=np.float32))
    mk = np.ascontiguousarray(np.asarray(mask, dtype=np.float32))
    nc = _get_program()
    in_maps = _make_in_maps(fg, mk)

    from concourse.bass_utils import run_bass_kernel_spmd
    res = run_bass_kernel_spmd(nc, in_maps, core_ids=list(range(8)))

    out = np.empty((4, C, L), np.float32)
    for core in range(8):
        b, h = core // 2, core % 2
        # kernel columns = rotated positions 1..32 = image rows h*32..h*32+31
        out[b][:, h * HALF:(h + 1) * HALF] = res.results[core]["out"]
    return out.reshape(4, C, 64, 64)
